# revision 22
# baseline (speedup 1.0000x reference)
"""2-layer GCN (GCNConv -> LeakyReLU -> GCNConv) on 8 Trainium2 NeuronCores.

Strategy: dst-partition the graph across 8 cores (each core owns N/8
destination rows and all edges pointing into them). Every core computes the
full dense h = x @ W.T (replicated, cheap), writes it row-major to its local
HBM, bulk-gathers h[src] for its edges with dma_gather (int16 indices; the
node table is split in two halves so indices fit in int16), and aggregates
with norm-weighted one-hot matmuls accumulated in PSUM. Self-loops are
materialized as explicit edges on the host; bias is folded in as one extra
matmul per block. Between layers the per-core activations are PE-transposed
and AllGathered so layer 2 can consume them directly as matmul lhsT.
"""

import math

import numpy as np
import ml_dtypes

from concourse import bacc, bass, mybir
import concourse.tile as tile

BF16 = mybir.dt.bfloat16
F32 = mybir.dt.float32
I16 = mybir.dt.int16

NCORES = 8
D = 128
NEG_SLOPE = 0.01
import os as _os
GCMAX = int(_os.environ.get("GCN_GCMAX", "96"))  # max chunks per dma_gather call
# debug toggles (produce wrong results; only for isolating device hangs)
_NO_AG = bool(int(_os.environ.get("GCN_NO_AG", "0")))
_NO_GATHER = bool(int(_os.environ.get("GCN_NO_GATHER", "0")))
_NO_TP = bool(int(_os.environ.get("GCN_NO_TP", "0")))
# emit only a prefix of the pipeline: 1=h1, 2=+edge1, 3=+AG, 4=+h2, 5=full
_PHASES = int(_os.environ.get("GCN_PHASES", "5"))
# edge-phase content level for debug: gather | pb | mm | full
_EDGE_SUB = _os.environ.get("GCN_EDGE_SUB", "full")
HGROUP = 8  # h-compute blocks per DMA group


class Plan:
    pass


def make_plan(n_nodes, edge_index):
    """Host-side graph preprocessing: padding, degrees/norms, self-loop
    edges, per-core dst-partitioned + per-(block,half) chunked edge slots."""
    p = Plan()
    src = edge_index[0].astype(np.int64)
    dst = edge_index[1].astype(np.int64)

    unit = NCORES * 128
    p.N = n_nodes
    p.NPAD = ((n_nodes + unit - 1) // unit) * unit
    p.PCN = p.NPAD // NCORES
    p.B = p.PCN // 128
    p.HALF = p.NPAD // 2
    assert p.HALF - 1 <= 32767, "node count too large for int16 half-split"

    deg = np.bincount(dst, minlength=p.NPAD).astype(np.float32) + 1.0
    dis = (1.0 / np.sqrt(deg)).astype(np.float32)
    p.dis = dis

    # append self-loop edges (norm = dis^2), matching the reference's
    # analytic self-loop term
    alln = np.arange(p.NPAD, dtype=np.int64)
    src_a = np.concatenate([src, alln])
    dst_a = np.concatenate([dst, alln])
    norm_a = np.concatenate([dis[src] * dis[dst], dis * dis]).astype(np.float32)

    core = dst_a // p.PCN
    lb = (dst_a % p.PCN) // 128
    dloc = (dst_a % 128).astype(np.float32)
    halfbit = (src_a >= p.HALF).astype(np.int64)
    seg = (core * p.B + lb) * 2 + halfbit
    nseg = NCORES * p.B * 2

    order = np.lexsort((src_a, seg))
    seg_s = seg[order]
    src_s = src_a[order]
    dloc_s = dloc[order]
    norm_s = norm_a[order]

    counts = np.bincount(seg_s, minlength=nseg)
    cnt = counts.reshape(NCORES, p.B, 2)
    # per-(block,half) chunk counts, shared across cores (max over cores)
    p.chl = [max(1, int(math.ceil(cnt[:, b, 0].max() / 128))) for b in range(p.B)]
    p.chh = [max(1, int(math.ceil(cnt[:, b, 1].max() / 128))) for b in range(p.B)]
    p.SLch = sum(p.chl)
    p.SHch = sum(p.chh)
    p.NCH = p.SLch + p.SHch
    p.STOT = p.NCH * 128
    p.lofs = np.concatenate([[0], np.cumsum(p.chl)])[:-1]  # chunk offset of block b in L
    p.hofs = p.SLch + np.concatenate([[0], np.cumsum(p.chh)])[:-1]

    # slot base for each seg id
    segid = np.arange(nseg)
    sblk = (segid // 2) % p.B
    sh = segid % 2
    base = np.where(sh == 0, p.lofs[sblk] * 128, p.hofs[sblk] * 128)

    seg_starts = np.zeros(nseg + 1, np.int64)
    np.cumsum(counts, out=seg_starts[1:])
    rank = np.arange(len(seg_s)) - seg_starts[seg_s]
    slot = base[seg_s] + rank
    corefor = seg_s // (2 * p.B)

    idx_all = np.zeros((NCORES, p.STOT), np.int32)
    dl_all = np.zeros((NCORES, p.STOT), np.float32)
    nm_all = np.zeros((NCORES, p.STOT), np.float32)
    val = np.where(src_s >= p.HALF, src_s - p.HALF, src_s)
    idx_all[corefor, slot] = val
    dl_all[corefor, slot] = dloc_s
    nm_all[corefor, slot] = norm_s

    # dma_gather index layout: [128, STOT/16] int16, slot s at [s%16, s//16],
    # replicated across the 8 groups of 16 partitions
    idx16 = idx_all.astype(np.int16).reshape(NCORES, p.STOT // 16, 16)
    idx16 = np.ascontiguousarray(idx16.transpose(0, 2, 1))
    p.idx16 = np.ascontiguousarray(np.tile(idx16, (1, 8, 1)))
    # per-chunk metadata, [128, NCH] with column = chunk
    p.dl = np.ascontiguousarray(dl_all.reshape(NCORES, p.NCH, 128).transpose(0, 2, 1))
    p.nm = np.ascontiguousarray(nm_all.reshape(NCORES, p.NCH, 128).transpose(0, 2, 1))

    # gather call plan: (is_h, chunk_off_in_global_chunkspace, nchunks)
    p.calls = []
    for is_h, n_region, off in ((0, p.SLch, 0), (1, p.SHch, p.SLch)):
        nc_calls = max(1, math.ceil(n_region / GCMAX))
        per = math.ceil(n_region / nc_calls)
        c0 = 0
        while c0 < n_region:
            cn = min(per, n_region - c0)
            p.calls.append((is_h, off + c0, cn))
            c0 += cn
    # chunk -> (call index, local column)
    p.chunk_call = np.zeros((p.NCH, 2), np.int64)
    for gi, (_, coff, cn) in enumerate(p.calls):
        for c in range(cn):
            p.chunk_call[coff + c] = (gi, c)

    p.key = (p.NPAD, p.B, tuple(p.chl), tuple(p.chh))
    return p


def make_in_maps(plan, x, W1, b1, W2, b2):
    p = plan
    xpad = np.zeros((p.NPAD, D), np.float32)
    xpad[: p.N] = x
    xT = np.ascontiguousarray(xpad.T).astype(ml_dtypes.bfloat16)

    iota = np.tile(np.arange(128, dtype=np.float32)[None, :], (128, 1))
    ident = np.eye(128, dtype=np.float32)
    oneh = np.zeros((128, 128), np.float32)
    oneh[0, :] = 1.0
    br1 = np.zeros((128, 128), np.float32)
    br1[0, :] = b1
    br2 = np.zeros((128, 128), np.float32)
    br2[0, :] = b2

    common = {
        "xT": xT,
        "w1t": np.ascontiguousarray(W1.T).astype(ml_dtypes.bfloat16),
        "w2t": np.ascontiguousarray(W2.T).astype(ml_dtypes.bfloat16),
        "iota": iota.astype(ml_dtypes.bfloat16),
        "ident": ident.astype(ml_dtypes.bfloat16),
        "oneh": oneh.astype(ml_dtypes.bfloat16),
        "br1": br1.astype(ml_dtypes.bfloat16),
        "br2": br2.astype(ml_dtypes.bfloat16),
    }
    return [
        dict(common, idx=p.idx16[k], dln=p.dl[k], nmn=p.nm[k]) for k in range(NCORES)
    ]


def build_program(plan):
    p = plan
    NB = p.NPAD // 128

    nc = bacc.Bacc(
        "TRN2", target_bir_lowering=False, debug=False, num_devices=NCORES
    )

    xT_d = nc.dram_tensor("xT", [128, p.NPAD], BF16, kind="ExternalInput")
    w1t_d = nc.dram_tensor("w1t", [128, 128], BF16, kind="ExternalInput")
    w2t_d = nc.dram_tensor("w2t", [128, 128], BF16, kind="ExternalInput")
    iota_d = nc.dram_tensor("iota", [128, 128], BF16, kind="ExternalInput")
    ident_d = nc.dram_tensor("ident", [128, 128], BF16, kind="ExternalInput")
    oneh_d = nc.dram_tensor("oneh", [128, 128], BF16, kind="ExternalInput")
    br1_d = nc.dram_tensor("br1", [128, 128], BF16, kind="ExternalInput")
    br2_d = nc.dram_tensor("br2", [128, 128], BF16, kind="ExternalInput")
    idx_d = nc.dram_tensor("idx", [128, p.STOT // 16], I16, kind="ExternalInput")
    dln_d = nc.dram_tensor("dln", [128, p.NCH], F32, kind="ExternalInput")
    nmn_d = nc.dram_tensor("nmn", [128, p.NCH], F32, kind="ExternalInput")
    out_d = nc.dram_tensor("out", [p.PCN, 128], F32, kind="ExternalOutput")

    with tile.TileContext(nc) as tc:
        with (
            tc.tile_pool(name="dram", bufs=1, space="DRAM") as dpool,
            tc.tile_pool(name="const", bufs=1) as cpool,
            tc.tile_pool(name="work", bufs=2) as wpool,
            tc.tile_pool(name="psum", bufs=2, space="PSUM") as pspool,
        ):
            h1_t = dpool.tile([p.NPAD, 128], BF16, name="h1buf")
            h2_t = dpool.tile([p.NPAD, 128], BF16, name="h2buf")
            agin_t = dpool.tile([128, p.PCN], BF16, name="aginbuf")
            # NOTE: addr_space="Shared" would be faster for the collective,
            # but neuronxcc's DataLocalityOpt crashes on DMA loads from
            # Shared scratchpad tensors, so keep it Local.
            agout_t = dpool.tile([NCORES * 128, p.PCN], BF16, name="agoutbuf")

            def cload(dram, shape, dtype, name):
                t = cpool.tile(shape, dtype, name=name)
                nc.sync.dma_start(out=t[:], in_=dram.ap())
                return t

            w1t_s = cload(w1t_d, [128, 128], BF16, "w1t_s")
            w2t_s = cload(w2t_d, [128, 128], BF16, "w2t_s")
            iota_s = cload(iota_d, [128, 128], BF16, "iota_s")
            ident_s = cload(ident_d, [128, 128], BF16, "ident_s")
            oneh_s = cload(oneh_d, [128, 128], BF16, "oneh_s")
            br1_s = cload(br1_d, [128, 128], BF16, "br1_s")
            br2_s = cload(br2_d, [128, 128], BF16, "br2_s")
            idx_s = cload(idx_d, [128, p.STOT // 16], I16, "idx_s")
            dln_s = cload(dln_d, [128, p.NCH], F32, "dln_s")
            nmn_s = cload(nmn_d, [128, p.NCH], F32, "nmn_s")

            def h_phase(layer):
                """h = lhs @ W.T for all NPAD nodes, row-major bf16 to DRAM."""
                wst = w1t_s if layer == 1 else w2t_s
                hdst = h1_t if layer == 1 else h2_t
                groups = []
                if layer == 1:
                    nb0 = 0
                    while nb0 < NB:
                        gs = min(HGROUP, NB - nb0)
                        groups.append((nb0, gs))
                        nb0 += gs
                else:
                    # lhs tiles come from the AllGather output; groups must
                    # not cross rank boundaries
                    for r in range(NCORES):
                        lb0 = 0
                        while lb0 < p.B:
                            gs = min(HGROUP, p.B - lb0)
                            groups.append((r * p.B + lb0, gs))
                            lb0 += gs
                for gi, (nb0, gs) in enumerate(groups):
                    lhs_g = wpool.tile(
                        [128, HGROUP * 128], BF16, tag="lhsg", name="lhs_g"
                    )
                    if layer == 1:
                        nc.sync.dma_start(
                            out=lhs_g[:, : gs * 128],
                            in_=xT_d[:, nb0 * 128 : (nb0 + gs) * 128],
                        )
                    else:
                        r = nb0 // p.B
                        lb0 = nb0 - r * p.B
                        nc.sync.dma_start(
                            out=lhs_g[:, : gs * 128],
                            in_=agout_t[
                                r * 128 : (r + 1) * 128,
                                lb0 * 128 : (lb0 + gs) * 128,
                            ],
                        )
                    hst = wpool.tile(
                        [128, HGROUP, 128], BF16, tag="hst", bufs=3, name="hst"
                    )
                    for j in range(gs):
                        ps = pspool.tile(
                            [128, 128], F32, tag="hps", bufs=4, name="hps"
                        )
                        nc.tensor.matmul(
                            out=ps[:],
                            lhsT=lhs_g[:, j * 128 : (j + 1) * 128],
                            rhs=wst[:],
                            start=True,
                            stop=True,
                        )
                        if j % 2 == 0:
                            nc.scalar.copy(out=hst[:, j, :], in_=ps[:])
                        else:
                            nc.vector.tensor_copy(out=hst[:, j, :], in_=ps[:])
                    nc.sync.dma_start(
                        out=hdst[nb0 * 128 : (nb0 + gs) * 128, :].rearrange(
                            "(j q) f -> q j f", q=128
                        ),
                        in_=hst[:, :gs, :],
                    )

            def edge_phase(layer):
                htab = h1_t if layer == 1 else h2_t
                brs = br1_s if layer == 1 else br2_s
                # bulk gathers of h[src] for this core's edge slots. The L
                # and H regions get separate buffer tags: a block consumes an
                # early L call together with a late H call, so rotating both
                # regions through one tag deadlocks the slot rotation.
                call_tiles = []
                for gi, (is_h, coff, cn) in enumerate(
                    [] if _EDGE_SUB == "none" else p.calls
                ):
                    gt = wpool.tile(
                        [128, GCMAX, 128],
                        BF16,
                        tag=("gbufH" if is_h else "gbufL"),
                        name="gt",
                    )
                    tab = (
                        htab[p.HALF :, :] if is_h else htab[: p.HALF, :]
                    )
                    ni = cn * 128
                    soff = coff * 128
                    if _NO_GATHER:
                        nc.vector.memset(gt[:, :cn, :], 0)
                    else:
                        nc.gpsimd.dma_gather(
                            gt[:, :cn, :],
                            tab,
                            idx_s[:, soff // 16 : (soff + ni) // 16],
                            ni,
                            ni,
                            128,
                            elem_step=128,
                            single_packet=(ni <= 1024),
                        )
                    call_tiles.append(gt)

                debug_anchor = layer == 1 and _PHASES == 2
                if layer == 1:
                    a1st = wpool.tile(
                        [128, p.B * 128], BF16, tag="a1st", bufs=1, name="a1st"
                    )
                if layer == 2 or debug_anchor:
                    outst = wpool.tile(
                        [128, p.B, 128], F32, tag="outst", bufs=1, name="outst"
                    )

                for b in range(p.B):
                    if _EDGE_SUB == "none":
                        st = wpool.tile(
                            [128, 128], F32, tag="nst", bufs=2, name="st"
                        )
                        nc.vector.memset(st[:], float(b))
                        nc.vector.tensor_copy(out=outst[:, b, :], in_=st[:])
                        continue
                    if _EDGE_SUB == "gather":
                        gi, c = p.chunk_call[p.lofs[b]]
                        nc.vector.tensor_copy(
                            out=outst[:, b, :], in_=call_tiles[gi][:, c : c + 1, :]
                        )
                        continue
                    if _EDGE_SUB == "pb":
                        sink = wpool.tile(
                            [128, 128], BF16, tag="sink", bufs=2, name="sink"
                        )
                        nc.vector.memset(sink[:], 0)
                        for ci in [p.lofs[b] + c for c in range(p.chl[b])] + [
                            p.hofs[b] + c for c in range(p.chh[b])
                        ]:
                            pt = wpool.tile(
                                [128, 128], BF16, tag="ptile", bufs=4, name="pt"
                            )
                            nc.vector.tensor_scalar(
                                pt[:],
                                iota_s[:],
                                dln_s[:, ci : ci + 1],
                                nmn_s[:, ci : ci + 1],
                                mybir.AluOpType.is_equal,
                                mybir.AluOpType.mult,
                            )
                            nc.vector.tensor_tensor(
                                out=sink[:], in0=sink[:], in1=pt[:],
                                op=mybir.AluOpType.max,
                            )
                        nc.vector.tensor_copy(out=outst[:, b, :], in_=sink[:])
                        continue
                    agg = pspool.tile([128, 128], F32, tag="agg", name="agg")
                    chunk_ids = [p.lofs[b] + c for c in range(p.chl[b])] + [
                        p.hofs[b] + c for c in range(p.chh[b])
                    ]
                    for k, ci in enumerate(chunk_ids):
                        pt = wpool.tile(
                            [128, 128], BF16, tag="ptile", bufs=4, name="pt"
                        )
                        nc.vector.tensor_scalar(
                            pt[:],
                            iota_s[:],
                            dln_s[:, ci : ci + 1],
                            nmn_s[:, ci : ci + 1],
                            mybir.AluOpType.is_equal,
                            mybir.AluOpType.mult,
                        )
                        gi, c = p.chunk_call[ci]
                        nc.tensor.matmul(
                            out=agg[:],
                            lhsT=pt[:],
                            rhs=call_tiles[gi][:, c : c + 1, :],
                            start=(k == 0),
                            stop=False,
                        )
                    # bias as one extra rank-1 matmul
                    nc.tensor.matmul(
                        out=agg[:], lhsT=oneh_s[:], rhs=brs[:], start=False, stop=True
                    )
                    if _EDGE_SUB == "mm":
                        nc.scalar.copy(out=outst[:, b, :], in_=agg[:])
                        continue
                    if layer == 1:
                        t1 = wpool.tile([128, 128], F32, tag="ep1", name="t1")
                        nc.vector.tensor_scalar(
                            t1[:],
                            agg[:],
                            NEG_SLOPE,
                            None,
                            mybir.AluOpType.mult,
                        )
                        a1b = wpool.tile([128, 128], BF16, tag="a1b", name="a1b")
                        nc.vector.tensor_tensor(
                            out=a1b[:], in0=agg[:], in1=t1[:], op=mybir.AluOpType.max
                        )
                        if debug_anchor:
                            nc.vector.tensor_copy(out=outst[:, b, :], in_=a1b[:])
                        if _NO_TP:
                            nc.vector.tensor_copy(
                                out=a1st[:, b * 128 : (b + 1) * 128], in_=a1b[:]
                            )
                        else:
                            tp = pspool.tile(
                                [128, 128], BF16, space="PSUM", tag="tp", name="tp"
                            )
                            nc.tensor.transpose(
                                out=tp[:], in_=a1b[:], identity=ident_s[:]
                            )
                            if b % 2 == 0:
                                nc.scalar.copy(
                                    out=a1st[:, b * 128 : (b + 1) * 128], in_=tp[:]
                                )
                            else:
                                nc.vector.tensor_copy(
                                    out=a1st[:, b * 128 : (b + 1) * 128], in_=tp[:]
                                )
                    else:
                        if b % 2 == 0:
                            nc.scalar.copy(out=outst[:, b, :], in_=agg[:])
                        else:
                            nc.vector.tensor_copy(out=outst[:, b, :], in_=agg[:])

                if layer == 1:
                    if debug_anchor:
                        nc.sync.dma_start(
                            out=out_d.ap().rearrange("(b q) f -> q b f", q=128),
                            in_=outst[:, :, :],
                        )
                    return a1st
                nc.sync.dma_start(
                    out=out_d.ap().rearrange("(b q) f -> q b f", q=128),
                    in_=outst[:, :, :],
                )
                return None

            def ag_phase(a1st):
                nc.sync.dma_start(out=agin_t[:, :], in_=a1st[:, :])
                if _NO_AG:
                    # debug stand-in: replicate own shard into all slots
                    # (SBUF -> DRAM; DRAM -> DRAM DMA is known-buggy)
                    for r in range(NCORES):
                        nc.sync.dma_start(
                            out=agout_t[r * 128 : (r + 1) * 128, :],
                            in_=a1st[:, :],
                        )
                else:
                    nc.gpsimd.collective_compute(
                        "AllGather",
                        mybir.AluOpType.bypass,
                        replica_groups=[list(range(NCORES))],
                        ins=[agin_t[:, :].opt()],
                        outs=[agout_t[:, :].opt()],
                    )

            h_phase(1)
            if _PHASES >= 2:
                a1st = edge_phase(1)
                if _PHASES >= 3:
                    ag_phase(a1st)
                    if _PHASES >= 4:
                        h_phase(2)
                        if _PHASES >= 5:
                            edge_phase(2)

    nc.compile()
    return nc


_CACHE = {}


def _get_program(plan):
    nc = _CACHE.get(plan.key)
    if nc is None:
        nc = build_program(plan)
        _CACHE[plan.key] = nc
    return nc


def kernel(x, edge_index, batch, W1, b1, W2, b2):
    from concourse.bass_utils import run_bass_kernel_spmd

    x = np.asarray(x, np.float32)
    edge_index = np.asarray(edge_index)
    plan = make_plan(x.shape[0], edge_index)
    in_maps = make_in_maps(
        plan,
        x,
        np.asarray(W1, np.float32),
        np.asarray(b1, np.float32),
        np.asarray(W2, np.float32),
        np.asarray(b2, np.float32),
    )
    nc = _get_program(plan)
    res = run_bass_kernel_spmd(nc, in_maps, core_ids=list(range(NCORES)))
    out = np.concatenate([res.results[k]["out"] for k in range(NCORES)], axis=0)
    return np.ascontiguousarray(out[: plan.N]).astype(np.float32)


# revision 25
# speedup vs baseline: 1.5085x; 1.5085x over previous
"""2-layer GCN (GCNConv -> LeakyReLU -> GCNConv) on 8 Trainium2 NeuronCores.

Strategy: dst-partition the graph across 8 cores (each core owns N/8
destination rows and all edges pointing into them). Every core computes the
full dense h = x @ W.T (replicated, cheap), writes it row-major to its local
HBM, bulk-gathers h[src] for its edges with dma_gather (int16 indices; the
node table is split in two halves so indices fit in int16), and aggregates
with norm-weighted one-hot matmuls accumulated in PSUM. Self-loops are
materialized as explicit edges on the host; bias is folded in as one extra
matmul per block. Between layers the per-core activations are PE-transposed
and AllGathered so layer 2 can consume them directly as matmul lhsT.
"""

import math

import numpy as np
import ml_dtypes

from concourse import bacc, bass, mybir
import concourse.tile as tile

BF16 = mybir.dt.bfloat16
F32 = mybir.dt.float32
I16 = mybir.dt.int16

NCORES = 8
D = 128
NEG_SLOPE = 0.01
import os as _os
GCMAX = int(_os.environ.get("GCN_GCMAX", "96"))  # max chunks per dma_gather call
# debug toggles (produce wrong results; only for isolating device hangs)
_NO_AG = bool(int(_os.environ.get("GCN_NO_AG", "0")))
_NO_GATHER = bool(int(_os.environ.get("GCN_NO_GATHER", "0")))
_NO_TP = bool(int(_os.environ.get("GCN_NO_TP", "0")))
# emit only a prefix of the pipeline: 1=h1, 2=+edge1, 3=+AG, 4=+h2, 5=full
_PHASES = int(_os.environ.get("GCN_PHASES", "5"))
# edge-phase content level for debug: gather | pb | mm | full
_EDGE_SUB = _os.environ.get("GCN_EDGE_SUB", "full")
_NQUEUES = int(_os.environ.get("GCN_NQUEUES", "1"))
HGROUP = 8  # h-compute blocks per DMA group


class Plan:
    pass


def make_plan(n_nodes, edge_index):
    """Host-side graph preprocessing: padding, degrees/norms, self-loop
    edges, per-core dst-partitioned + per-(block,half) chunked edge slots."""
    p = Plan()
    src = edge_index[0].astype(np.int64)
    dst = edge_index[1].astype(np.int64)

    unit = NCORES * 128
    p.N = n_nodes
    p.NPAD = ((n_nodes + unit - 1) // unit) * unit
    p.PCN = p.NPAD // NCORES
    p.B = p.PCN // 128
    p.HALF = p.NPAD // 2
    assert p.HALF - 1 <= 32767, "node count too large for int16 half-split"

    deg = np.bincount(dst, minlength=p.NPAD).astype(np.float32) + 1.0
    dis = (1.0 / np.sqrt(deg)).astype(np.float32)
    p.dis = dis

    # append self-loop edges (norm = dis^2), matching the reference's
    # analytic self-loop term
    alln = np.arange(p.NPAD, dtype=np.int64)
    src_a = np.concatenate([src, alln])
    dst_a = np.concatenate([dst, alln])
    norm_a = np.concatenate([dis[src] * dis[dst], dis * dis]).astype(np.float32)

    core = dst_a // p.PCN
    lb = (dst_a % p.PCN) // 128
    dloc = (dst_a % 128).astype(np.float32)
    halfbit = (src_a >= p.HALF).astype(np.int64)
    seg = (core * p.B + lb) * 2 + halfbit
    nseg = NCORES * p.B * 2

    order = np.lexsort((src_a, seg))
    seg_s = seg[order]
    src_s = src_a[order]
    dloc_s = dloc[order]
    norm_s = norm_a[order]

    counts = np.bincount(seg_s, minlength=nseg)
    cnt = counts.reshape(NCORES, p.B, 2)
    # per-(block,half) chunk counts, shared across cores (max over cores)
    p.chl = [max(1, int(math.ceil(cnt[:, b, 0].max() / 128))) for b in range(p.B)]
    p.chh = [max(1, int(math.ceil(cnt[:, b, 1].max() / 128))) for b in range(p.B)]
    p.SLch = sum(p.chl)
    p.SHch = sum(p.chh)
    p.NCH = p.SLch + p.SHch
    p.STOT = p.NCH * 128
    p.lofs = np.concatenate([[0], np.cumsum(p.chl)])[:-1]  # chunk offset of block b in L
    p.hofs = p.SLch + np.concatenate([[0], np.cumsum(p.chh)])[:-1]

    # slot base for each seg id
    segid = np.arange(nseg)
    sblk = (segid // 2) % p.B
    sh = segid % 2
    base = np.where(sh == 0, p.lofs[sblk] * 128, p.hofs[sblk] * 128)

    seg_starts = np.zeros(nseg + 1, np.int64)
    np.cumsum(counts, out=seg_starts[1:])
    rank = np.arange(len(seg_s)) - seg_starts[seg_s]
    slot = base[seg_s] + rank
    corefor = seg_s // (2 * p.B)

    idx_all = np.zeros((NCORES, p.STOT), np.int32)
    dl_all = np.zeros((NCORES, p.STOT), np.float32)
    nm_all = np.zeros((NCORES, p.STOT), np.float32)
    val = np.where(src_s >= p.HALF, src_s - p.HALF, src_s)
    idx_all[corefor, slot] = val
    dl_all[corefor, slot] = dloc_s
    nm_all[corefor, slot] = norm_s

    # dma_gather index layout: [128, STOT/16] int16, slot s at [s%16, s//16],
    # replicated across the 8 groups of 16 partitions
    idx16 = idx_all.astype(np.int16).reshape(NCORES, p.STOT // 16, 16)
    idx16 = np.ascontiguousarray(idx16.transpose(0, 2, 1))
    p.idx16 = np.ascontiguousarray(np.tile(idx16, (1, 8, 1)))
    # per-chunk metadata, [128, NCH] with column = chunk
    p.dl = np.ascontiguousarray(dl_all.reshape(NCORES, p.NCH, 128).transpose(0, 2, 1))
    p.nm = np.ascontiguousarray(nm_all.reshape(NCORES, p.NCH, 128).transpose(0, 2, 1))

    # gather call plan: (is_h, chunk_off_in_global_chunkspace, nchunks)
    p.calls = []
    for is_h, n_region, off in ((0, p.SLch, 0), (1, p.SHch, p.SLch)):
        nc_calls = max(1, math.ceil(n_region / GCMAX))
        per = math.ceil(n_region / nc_calls)
        c0 = 0
        while c0 < n_region:
            cn = min(per, n_region - c0)
            p.calls.append((is_h, off + c0, cn))
            c0 += cn
    # chunk -> (call index, local column)
    p.chunk_call = np.zeros((p.NCH, 2), np.int64)
    for gi, (_, coff, cn) in enumerate(p.calls):
        for c in range(cn):
            p.chunk_call[coff + c] = (gi, c)

    p.key = (p.NPAD, p.B, tuple(p.chl), tuple(p.chh))
    return p


def make_in_maps(plan, x, W1, b1, W2, b2):
    p = plan
    xpad = np.zeros((p.NPAD, D), np.float32)
    xpad[: p.N] = x
    xT = np.ascontiguousarray(xpad.T).astype(ml_dtypes.bfloat16)

    iota = np.tile(np.arange(128, dtype=np.float32)[None, :], (128, 1))
    ident = np.eye(128, dtype=np.float32)
    oneh = np.zeros((128, 128), np.float32)
    oneh[0, :] = 1.0
    br1 = np.zeros((128, 128), np.float32)
    br1[0, :] = b1
    br2 = np.zeros((128, 128), np.float32)
    br2[0, :] = b2

    common = {
        "xT": xT,
        "w1t": np.ascontiguousarray(W1.T).astype(ml_dtypes.bfloat16),
        "w2t": np.ascontiguousarray(W2.T).astype(ml_dtypes.bfloat16),
        "iota": iota.astype(ml_dtypes.bfloat16),
        "ident": ident.astype(ml_dtypes.bfloat16),
        "oneh": oneh.astype(ml_dtypes.bfloat16),
        "br1": br1.astype(ml_dtypes.bfloat16),
        "br2": br2.astype(ml_dtypes.bfloat16),
    }
    return [
        dict(common, idx=p.idx16[k], dln=p.dl[k], nmn=p.nm[k]) for k in range(NCORES)
    ]


def build_program(plan):
    p = plan
    NB = p.NPAD // 128

    nc = bacc.Bacc(
        "TRN2",
        target_bir_lowering=False,
        debug=False,
        num_devices=NCORES,
        num_swdge_queues=_NQUEUES,
    )

    xT_d = nc.dram_tensor("xT", [128, p.NPAD], BF16, kind="ExternalInput")
    w1t_d = nc.dram_tensor("w1t", [128, 128], BF16, kind="ExternalInput")
    w2t_d = nc.dram_tensor("w2t", [128, 128], BF16, kind="ExternalInput")
    iota_d = nc.dram_tensor("iota", [128, 128], BF16, kind="ExternalInput")
    ident_d = nc.dram_tensor("ident", [128, 128], BF16, kind="ExternalInput")
    oneh_d = nc.dram_tensor("oneh", [128, 128], BF16, kind="ExternalInput")
    br1_d = nc.dram_tensor("br1", [128, 128], BF16, kind="ExternalInput")
    br2_d = nc.dram_tensor("br2", [128, 128], BF16, kind="ExternalInput")
    idx_d = nc.dram_tensor("idx", [128, p.STOT // 16], I16, kind="ExternalInput")
    dln_d = nc.dram_tensor("dln", [128, p.NCH], F32, kind="ExternalInput")
    nmn_d = nc.dram_tensor("nmn", [128, p.NCH], F32, kind="ExternalInput")
    out_d = nc.dram_tensor("out", [p.PCN, 128], F32, kind="ExternalOutput")

    with tile.TileContext(nc) as tc:
        with (
            tc.tile_pool(name="dram", bufs=1, space="DRAM") as dpool,
            tc.tile_pool(name="const", bufs=1) as cpool,
            tc.tile_pool(name="work", bufs=2) as wpool,
            tc.tile_pool(name="psum", bufs=2, space="PSUM") as pspool,
        ):
            h1_t = dpool.tile([p.NPAD, 128], BF16, name="h1buf")
            h2_t = dpool.tile([p.NPAD, 128], BF16, name="h2buf")
            agin_t = dpool.tile([128, p.PCN], BF16, name="aginbuf")
            # NOTE: addr_space="Shared" would be faster for the collective,
            # but neuronxcc's DataLocalityOpt crashes on DMA loads from
            # Shared scratchpad tensors, so keep it Local.
            agout_t = dpool.tile([NCORES * 128, p.PCN], BF16, name="agoutbuf")

            def cload(dram, shape, dtype, name):
                t = cpool.tile(shape, dtype, name=name)
                nc.sync.dma_start(out=t[:], in_=dram.ap())
                return t

            w1t_s = cload(w1t_d, [128, 128], BF16, "w1t_s")
            w2t_s = cload(w2t_d, [128, 128], BF16, "w2t_s")
            iota_s = cload(iota_d, [128, 128], BF16, "iota_s")
            ident_s = cload(ident_d, [128, 128], BF16, "ident_s")
            oneh_s = cload(oneh_d, [128, 128], BF16, "oneh_s")
            br1_s = cload(br1_d, [128, 128], BF16, "br1_s")
            br2_s = cload(br2_d, [128, 128], BF16, "br2_s")
            idx_s = cload(idx_d, [128, p.STOT // 16], I16, "idx_s")
            dln_s = cload(dln_d, [128, p.NCH], F32, "dln_s")
            nmn_s = cload(nmn_d, [128, p.NCH], F32, "nmn_s")

            def h_phase(layer):
                """h = lhs @ W.T for all NPAD nodes, row-major bf16 to DRAM."""
                wst = w1t_s if layer == 1 else w2t_s
                hdst = h1_t if layer == 1 else h2_t
                groups = []
                if layer == 1:
                    nb0 = 0
                    while nb0 < NB:
                        gs = min(HGROUP, NB - nb0)
                        groups.append((nb0, gs))
                        nb0 += gs
                else:
                    # lhs tiles come from the AllGather output; groups must
                    # not cross rank boundaries
                    for r in range(NCORES):
                        lb0 = 0
                        while lb0 < p.B:
                            gs = min(HGROUP, p.B - lb0)
                            groups.append((r * p.B + lb0, gs))
                            lb0 += gs
                for gi, (nb0, gs) in enumerate(groups):
                    lhs_g = wpool.tile(
                        [128, HGROUP * 128], BF16, tag="lhsg", name="lhs_g"
                    )
                    if layer == 1:
                        nc.sync.dma_start(
                            out=lhs_g[:, : gs * 128],
                            in_=xT_d[:, nb0 * 128 : (nb0 + gs) * 128],
                        )
                    else:
                        r = nb0 // p.B
                        lb0 = nb0 - r * p.B
                        nc.sync.dma_start(
                            out=lhs_g[:, : gs * 128],
                            in_=agout_t[
                                r * 128 : (r + 1) * 128,
                                lb0 * 128 : (lb0 + gs) * 128,
                            ],
                        )
                    hst = wpool.tile(
                        [128, HGROUP, 128], BF16, tag="hst", bufs=3, name="hst"
                    )
                    for j in range(gs):
                        ps = pspool.tile(
                            [128, 128], F32, tag="hps", bufs=4, name="hps"
                        )
                        nc.tensor.matmul(
                            out=ps[:],
                            lhsT=lhs_g[:, j * 128 : (j + 1) * 128],
                            rhs=wst[:],
                            start=True,
                            stop=True,
                        )
                        if j % 2 == 0:
                            nc.scalar.copy(out=hst[:, j, :], in_=ps[:])
                        else:
                            nc.vector.tensor_copy(out=hst[:, j, :], in_=ps[:])
                    nc.sync.dma_start(
                        out=hdst[nb0 * 128 : (nb0 + gs) * 128, :].rearrange(
                            "(j q) f -> q j f", q=128
                        ),
                        in_=hst[:, :gs, :],
                    )

            def edge_phase(layer):
                htab = h1_t if layer == 1 else h2_t
                brs = br1_s if layer == 1 else br2_s
                # bulk gathers of h[src] for this core's edge slots. The L
                # and H regions get separate buffer tags: a block consumes an
                # early L call together with a late H call, so rotating both
                # regions through one tag deadlocks the slot rotation.
                call_tiles = []
                for gi, (is_h, coff, cn) in enumerate(
                    [] if _EDGE_SUB == "none" else p.calls
                ):
                    gt = wpool.tile(
                        [128, GCMAX, 128],
                        BF16,
                        tag=("gbufH" if is_h else "gbufL"),
                        name="gt",
                    )
                    tab = (
                        htab[p.HALF :, :] if is_h else htab[: p.HALF, :]
                    )
                    ni = cn * 128
                    soff = coff * 128
                    if _NO_GATHER:
                        nc.vector.memset(gt[:, :cn, :], 0)
                    else:
                        nc.gpsimd.dma_gather(
                            gt[:, :cn, :],
                            tab,
                            idx_s[:, soff // 16 : (soff + ni) // 16],
                            ni,
                            ni,
                            128,
                            elem_step=128,
                            single_packet=(ni <= 1024),
                            queue_num=gi % _NQUEUES,
                        )
                    call_tiles.append(gt)

                debug_anchor = layer == 1 and _PHASES == 2
                if layer == 1:
                    a1st = wpool.tile(
                        [128, p.B * 128], BF16, tag="a1st", bufs=1, name="a1st"
                    )
                if layer == 2 or debug_anchor:
                    outst = wpool.tile(
                        [128, p.B, 128], F32, tag="outst", bufs=1, name="outst"
                    )

                for b in range(p.B):
                    if _EDGE_SUB == "none":
                        st = wpool.tile(
                            [128, 128], F32, tag="nst", bufs=2, name="st"
                        )
                        nc.vector.memset(st[:], float(b))
                        nc.vector.tensor_copy(out=outst[:, b, :], in_=st[:])
                        continue
                    if _EDGE_SUB == "gather":
                        gi, c = p.chunk_call[p.lofs[b]]
                        nc.vector.tensor_copy(
                            out=outst[:, b, :], in_=call_tiles[gi][:, c : c + 1, :]
                        )
                        continue
                    if _EDGE_SUB == "pb":
                        sink = wpool.tile(
                            [128, 128], BF16, tag="sink", bufs=2, name="sink"
                        )
                        nc.vector.memset(sink[:], 0)
                        for ci in [p.lofs[b] + c for c in range(p.chl[b])] + [
                            p.hofs[b] + c for c in range(p.chh[b])
                        ]:
                            pt = wpool.tile(
                                [128, 128], BF16, tag="ptile", bufs=4, name="pt"
                            )
                            nc.vector.tensor_scalar(
                                pt[:],
                                iota_s[:],
                                dln_s[:, ci : ci + 1],
                                nmn_s[:, ci : ci + 1],
                                mybir.AluOpType.is_equal,
                                mybir.AluOpType.mult,
                            )
                            nc.vector.tensor_tensor(
                                out=sink[:], in0=sink[:], in1=pt[:],
                                op=mybir.AluOpType.max,
                            )
                        nc.vector.tensor_copy(out=outst[:, b, :], in_=sink[:])
                        continue
                    agg = pspool.tile([128, 128], F32, tag="agg", name="agg")
                    chunk_ids = [p.lofs[b] + c for c in range(p.chl[b])] + [
                        p.hofs[b] + c for c in range(p.chh[b])
                    ]
                    for k, ci in enumerate(chunk_ids):
                        pt = wpool.tile(
                            [128, 128], BF16, tag="ptile", bufs=4, name="pt"
                        )
                        nc.vector.tensor_scalar(
                            pt[:],
                            iota_s[:],
                            dln_s[:, ci : ci + 1],
                            nmn_s[:, ci : ci + 1],
                            mybir.AluOpType.is_equal,
                            mybir.AluOpType.mult,
                        )
                        gi, c = p.chunk_call[ci]
                        nc.tensor.matmul(
                            out=agg[:],
                            lhsT=pt[:],
                            rhs=call_tiles[gi][:, c : c + 1, :],
                            start=(k == 0),
                            stop=False,
                        )
                    # bias as one extra rank-1 matmul
                    nc.tensor.matmul(
                        out=agg[:], lhsT=oneh_s[:], rhs=brs[:], start=False, stop=True
                    )
                    if _EDGE_SUB == "mm":
                        nc.scalar.copy(out=outst[:, b, :], in_=agg[:])
                        continue
                    if layer == 1:
                        t1 = wpool.tile([128, 128], F32, tag="ep1", name="t1")
                        nc.vector.tensor_scalar(
                            t1[:],
                            agg[:],
                            NEG_SLOPE,
                            None,
                            mybir.AluOpType.mult,
                        )
                        a1b = wpool.tile([128, 128], BF16, tag="a1b", name="a1b")
                        nc.vector.tensor_tensor(
                            out=a1b[:], in0=agg[:], in1=t1[:], op=mybir.AluOpType.max
                        )
                        if debug_anchor:
                            nc.vector.tensor_copy(out=outst[:, b, :], in_=a1b[:])
                        if _NO_TP:
                            nc.vector.tensor_copy(
                                out=a1st[:, b * 128 : (b + 1) * 128], in_=a1b[:]
                            )
                        else:
                            tp = pspool.tile(
                                [128, 128], BF16, space="PSUM", tag="tp", name="tp"
                            )
                            nc.tensor.transpose(
                                out=tp[:], in_=a1b[:], identity=ident_s[:]
                            )
                            if b % 2 == 0:
                                nc.scalar.copy(
                                    out=a1st[:, b * 128 : (b + 1) * 128], in_=tp[:]
                                )
                            else:
                                nc.vector.tensor_copy(
                                    out=a1st[:, b * 128 : (b + 1) * 128], in_=tp[:]
                                )
                    else:
                        if b % 2 == 0:
                            nc.scalar.copy(out=outst[:, b, :], in_=agg[:])
                        else:
                            nc.vector.tensor_copy(out=outst[:, b, :], in_=agg[:])

                if layer == 1:
                    if debug_anchor:
                        nc.sync.dma_start(
                            out=out_d.ap().rearrange("(b q) f -> q b f", q=128),
                            in_=outst[:, :, :],
                        )
                    return a1st
                nc.sync.dma_start(
                    out=out_d.ap().rearrange("(b q) f -> q b f", q=128),
                    in_=outst[:, :, :],
                )
                return None

            def ag_phase(a1st):
                nc.sync.dma_start(out=agin_t[:, :], in_=a1st[:, :])
                if _NO_AG:
                    # debug stand-in: replicate own shard into all slots
                    # (SBUF -> DRAM; DRAM -> DRAM DMA is known-buggy)
                    for r in range(NCORES):
                        nc.sync.dma_start(
                            out=agout_t[r * 128 : (r + 1) * 128, :],
                            in_=a1st[:, :],
                        )
                else:
                    nc.gpsimd.collective_compute(
                        "AllGather",
                        mybir.AluOpType.bypass,
                        replica_groups=[list(range(NCORES))],
                        ins=[agin_t[:, :].opt()],
                        outs=[agout_t[:, :].opt()],
                    )

            h_phase(1)
            if _PHASES >= 2:
                a1st = edge_phase(1)
                if _PHASES >= 3:
                    ag_phase(a1st)
                    if _PHASES >= 4:
                        h_phase(2)
                        if _PHASES >= 5:
                            edge_phase(2)

    nc.compile()
    return nc


_CACHE = {}


def _get_program(plan):
    nc = _CACHE.get(plan.key)
    if nc is None:
        nc = build_program(plan)
        _CACHE[plan.key] = nc
    return nc


def kernel(x, edge_index, batch, W1, b1, W2, b2):
    from concourse.bass_utils import run_bass_kernel_spmd

    x = np.asarray(x, np.float32)
    edge_index = np.asarray(edge_index)
    plan = make_plan(x.shape[0], edge_index)
    in_maps = make_in_maps(
        plan,
        x,
        np.asarray(W1, np.float32),
        np.asarray(b1, np.float32),
        np.asarray(W2, np.float32),
        np.asarray(b2, np.float32),
    )
    nc = _get_program(plan)
    res = run_bass_kernel_spmd(nc, in_maps, core_ids=list(range(NCORES)))
    out = np.concatenate([res.results[k]["out"] for k in range(NCORES)], axis=0)
    return np.ascontiguousarray(out[: plan.N]).astype(np.float32)


# revision 26
# speedup vs baseline: 1.8721x; 1.2410x over previous
"""2-layer GCN (GCNConv -> LeakyReLU -> GCNConv) on 8 Trainium2 NeuronCores.

Strategy: dst-partition the graph across 8 cores (each core owns N/8
destination rows and all edges pointing into them). Every core computes the
full dense h' = (x @ W.T) * dis[row] (replicated, cheap; dis = deg^-1/2 is
folded into the PSUM->SBUF copy), writes it row-major bf16 to local HBM,
bulk-gathers h'[src] for its edges with dma_gather (int16 indices; the node
table is split in two halves so indices fit in int16), and aggregates with
0/1 one-hot matmuls accumulated in PSUM; the dst-side dis[dst] scale and the
bias are applied per 128-row block in the epilogue. The symmetric norm
dis[src]*dis[dst] therefore never appears on the per-edge path. Self-loops
are materialized as explicit edges on the host. Between layers the per-core
activations are PE-transposed and AllGathered so layer 2 consumes them
directly as matmul lhsT.
"""

import math
import os as _os

import numpy as np
import ml_dtypes

from concourse import bacc, bass, mybir
import concourse.tile as tile

BF16 = mybir.dt.bfloat16
F32 = mybir.dt.float32
I16 = mybir.dt.int16

NCORES = 8
D = 128
NEG_SLOPE = 0.01
GCMAX = int(_os.environ.get("GCN_GCMAX", "8"))  # chunks per dma_gather call
_NQUEUES = int(_os.environ.get("GCN_NQUEUES", "4"))
_GBUFS = int(_os.environ.get("GCN_GBUFS", "8"))  # gather tile bufs per region
HGROUP = 8  # h-compute blocks per DMA group


class Plan:
    pass


def make_plan(n_nodes, edge_index):
    """Host-side graph preprocessing: padding, degrees, self-loop edges,
    per-core dst-partitioned + per-(block,half) chunked edge slots."""
    p = Plan()
    src = edge_index[0].astype(np.int64)
    dst = edge_index[1].astype(np.int64)

    unit = NCORES * 128
    p.N = n_nodes
    p.NPAD = ((n_nodes + unit - 1) // unit) * unit
    p.PCN = p.NPAD // NCORES
    p.B = p.PCN // 128
    p.NB = p.NPAD // 128
    p.HALF = p.NPAD // 2
    assert p.HALF - 1 <= 32767, "node count too large for int16 half-split"

    deg = np.bincount(dst, minlength=p.NPAD).astype(np.float32) + 1.0
    dis = (1.0 / np.sqrt(deg)).astype(np.float32)
    p.dis = dis

    # self-loop edges: with the separable norm, a self edge at (i, i) with a
    # 0/1 one-hot contributes dis[i]*h[i]*dis[i] = the reference's analytic
    # self-loop term
    alln = np.arange(p.NPAD, dtype=np.int64)
    src_a = np.concatenate([src, alln])
    dst_a = np.concatenate([dst, alln])

    core = dst_a // p.PCN
    lb = (dst_a % p.PCN) // 128
    dloc = (dst_a % 128).astype(np.float32)
    halfbit = (src_a >= p.HALF).astype(np.int64)
    seg = (core * p.B + lb) * 2 + halfbit
    nseg = NCORES * p.B * 2

    order = np.lexsort((src_a, seg))
    seg_s = seg[order]
    src_s = src_a[order]
    dloc_s = dloc[order]

    counts = np.bincount(seg_s, minlength=nseg)
    cnt = counts.reshape(NCORES, p.B, 2)
    # per-(block,half) chunk counts, shared across cores (max over cores)
    p.chl = [max(1, int(math.ceil(cnt[:, b, 0].max() / 128))) for b in range(p.B)]
    p.chh = [max(1, int(math.ceil(cnt[:, b, 1].max() / 128))) for b in range(p.B)]
    p.SLch = sum(p.chl)
    p.SHch = sum(p.chh)
    p.NCH = p.SLch + p.SHch
    p.STOT = p.NCH * 128
    p.lofs = np.concatenate([[0], np.cumsum(p.chl)])[:-1]
    p.hofs = p.SLch + np.concatenate([[0], np.cumsum(p.chh)])[:-1]

    segid = np.arange(nseg)
    sblk = (segid // 2) % p.B
    sh = segid % 2
    base = np.where(sh == 0, p.lofs[sblk] * 128, p.hofs[sblk] * 128)

    seg_starts = np.zeros(nseg + 1, np.int64)
    np.cumsum(counts, out=seg_starts[1:])
    rank = np.arange(len(seg_s)) - seg_starts[seg_s]
    slot = base[seg_s] + rank
    corefor = seg_s // (2 * p.B)

    idx_all = np.zeros((NCORES, p.STOT), np.int32)
    # pad slots keep dst_local = -1 so is_equal(iota, -1) zeroes their column
    dl_all = np.full((NCORES, p.STOT), -1.0, np.float32)
    val = np.where(src_s >= p.HALF, src_s - p.HALF, src_s)
    idx_all[corefor, slot] = val
    dl_all[corefor, slot] = dloc_s

    # dma_gather index layout: [128, STOT/16] int16, slot s at [s%16, s//16],
    # replicated across the 8 groups of 16 partitions
    idx16 = idx_all.astype(np.int16).reshape(NCORES, p.STOT // 16, 16)
    idx16 = np.ascontiguousarray(idx16.transpose(0, 2, 1))
    p.idx16 = np.ascontiguousarray(np.tile(idx16, (1, 8, 1)))
    # per-chunk dst_local metadata, [128, NCH] with column = chunk
    p.dl = np.ascontiguousarray(dl_all.reshape(NCORES, p.NCH, 128).transpose(0, 2, 1))

    # per-node dis in device layouts
    p.disn = np.ascontiguousarray(
        dis.reshape(p.NB, 128).T
    )  # [128, NB], node nb*128+q at [q, nb]
    p.diso = np.ascontiguousarray(
        dis.reshape(NCORES, p.B, 128).transpose(0, 2, 1)
    )  # [C, 128, B]

    # gather call plan: (is_h, chunk_off_in_global_chunkspace, nchunks)
    p.calls = []
    for is_h, n_region, off in ((0, p.SLch, 0), (1, p.SHch, p.SLch)):
        nc_calls = max(1, math.ceil(n_region / GCMAX))
        per = math.ceil(n_region / nc_calls)
        c0 = 0
        while c0 < n_region:
            cn = min(per, n_region - c0)
            p.calls.append((is_h, off + c0, cn))
            c0 += cn
    p.chunk_call = np.zeros((p.NCH, 2), np.int64)
    for gi, (_, coff, cn) in enumerate(p.calls):
        for c in range(cn):
            p.chunk_call[coff + c] = (gi, c)

    p.key = (p.NPAD, p.B, tuple(p.chl), tuple(p.chh))
    return p


def make_in_maps(plan, x, W1, b1, W2, b2):
    p = plan
    xpad = np.zeros((p.NPAD, D), np.float32)
    xpad[: p.N] = x
    xT = np.ascontiguousarray(xpad.T).astype(ml_dtypes.bfloat16)

    iota = np.tile(np.arange(128, dtype=np.float32)[None, :], (128, 1))
    ident = np.eye(128, dtype=np.float32)

    common = {
        "xT": xT,
        "w1t": np.ascontiguousarray(W1.T).astype(ml_dtypes.bfloat16),
        "w2t": np.ascontiguousarray(W2.T).astype(ml_dtypes.bfloat16),
        "iota": iota.astype(ml_dtypes.bfloat16),
        "ident": ident.astype(ml_dtypes.bfloat16),
        "bias1": np.ascontiguousarray(
            np.tile(np.asarray(b1, np.float32)[None, :], (128, 1))
        ),
        "bias2": np.ascontiguousarray(
            np.tile(np.asarray(b2, np.float32)[None, :], (128, 1))
        ),
        "disn": p.disn,
    }
    return [
        dict(common, idx=p.idx16[k], dln=p.dl[k], diso=p.diso[k])
        for k in range(NCORES)
    ]


def build_program(plan):
    p = plan

    nc = bacc.Bacc(
        "TRN2",
        target_bir_lowering=False,
        debug=False,
        num_devices=NCORES,
        num_swdge_queues=_NQUEUES,
    )

    xT_d = nc.dram_tensor("xT", [128, p.NPAD], BF16, kind="ExternalInput")
    w1t_d = nc.dram_tensor("w1t", [128, 128], BF16, kind="ExternalInput")
    w2t_d = nc.dram_tensor("w2t", [128, 128], BF16, kind="ExternalInput")
    iota_d = nc.dram_tensor("iota", [128, 128], BF16, kind="ExternalInput")
    ident_d = nc.dram_tensor("ident", [128, 128], BF16, kind="ExternalInput")
    bias1_d = nc.dram_tensor("bias1", [128, 128], F32, kind="ExternalInput")
    bias2_d = nc.dram_tensor("bias2", [128, 128], F32, kind="ExternalInput")
    disn_d = nc.dram_tensor("disn", [128, p.NB], F32, kind="ExternalInput")
    diso_d = nc.dram_tensor("diso", [128, p.B], F32, kind="ExternalInput")
    idx_d = nc.dram_tensor("idx", [128, p.STOT // 16], I16, kind="ExternalInput")
    dln_d = nc.dram_tensor("dln", [128, p.NCH], F32, kind="ExternalInput")
    out_d = nc.dram_tensor("out", [p.PCN, 128], F32, kind="ExternalOutput")

    with tile.TileContext(nc) as tc:
        with (
            tc.tile_pool(name="dram", bufs=1, space="DRAM") as dpool,
            tc.tile_pool(name="const", bufs=1) as cpool,
            tc.tile_pool(name="work", bufs=2) as wpool,
            tc.tile_pool(name="psum", bufs=2, space="PSUM") as pspool,
        ):
            h1_t = dpool.tile([p.NPAD, 128], BF16, name="h1buf")
            h2_t = dpool.tile([p.NPAD, 128], BF16, name="h2buf")
            agin_t = dpool.tile([128, p.PCN], BF16, name="aginbuf")
            # (addr_space="Shared" would be faster for the collective, but
            # neuronxcc's DataLocalityOpt crashes on DMA loads from Shared
            # scratchpad tensors)
            agout_t = dpool.tile([NCORES * 128, p.PCN], BF16, name="agoutbuf")

            def cload(dram, shape, dtype, name):
                t = cpool.tile(shape, dtype, name=name)
                nc.sync.dma_start(out=t[:], in_=dram.ap())
                return t

            w1t_s = cload(w1t_d, [128, 128], BF16, "w1t_s")
            w2t_s = cload(w2t_d, [128, 128], BF16, "w2t_s")
            iota_s = cload(iota_d, [128, 128], BF16, "iota_s")
            ident_s = cload(ident_d, [128, 128], BF16, "ident_s")
            bias1_s = cload(bias1_d, [128, 128], F32, "bias1_s")
            bias2_s = cload(bias2_d, [128, 128], F32, "bias2_s")
            disn_s = cload(disn_d, [128, p.NB], F32, "disn_s")
            diso_s = cload(diso_d, [128, p.B], F32, "diso_s")
            idx_s = cload(idx_d, [128, p.STOT // 16], I16, "idx_s")
            dln_s = cload(dln_d, [128, p.NCH], F32, "dln_s")

            def h_phase(layer):
                """h' = (lhs @ W.T) * dis[row] for all NPAD nodes -> DRAM."""
                wst = w1t_s if layer == 1 else w2t_s
                hdst = h1_t if layer == 1 else h2_t
                groups = []
                if layer == 1:
                    nb0 = 0
                    while nb0 < p.NB:
                        gs = min(HGROUP, p.NB - nb0)
                        groups.append((nb0, gs))
                        nb0 += gs
                else:
                    # lhs tiles come from the AllGather output; groups must
                    # not cross rank boundaries
                    for r in range(NCORES):
                        lb0 = 0
                        while lb0 < p.B:
                            gs = min(HGROUP, p.B - lb0)
                            groups.append((r * p.B + lb0, gs))
                            lb0 += gs
                for nb0, gs in groups:
                    lhs_g = wpool.tile(
                        [128, HGROUP * 128], BF16, tag="lhsg", name="lhs_g"
                    )
                    if layer == 1:
                        nc.sync.dma_start(
                            out=lhs_g[:, : gs * 128],
                            in_=xT_d[:, nb0 * 128 : (nb0 + gs) * 128],
                        )
                    else:
                        r = nb0 // p.B
                        lb0 = nb0 - r * p.B
                        nc.sync.dma_start(
                            out=lhs_g[:, : gs * 128],
                            in_=agout_t[
                                r * 128 : (r + 1) * 128,
                                lb0 * 128 : (lb0 + gs) * 128,
                            ],
                        )
                    hst = wpool.tile(
                        [128, HGROUP, 128], BF16, tag="hst", bufs=3, name="hst"
                    )
                    for j in range(gs):
                        nb = nb0 + j
                        ps = pspool.tile(
                            [128, 128], F32, tag="hps", bufs=4, name="hps"
                        )
                        nc.tensor.matmul(
                            out=ps[:],
                            lhsT=lhs_g[:, j * 128 : (j + 1) * 128],
                            rhs=wst[:],
                            start=True,
                            stop=True,
                        )
                        # PSUM -> SBUF copy doubles as the dis[row] scale
                        if j % 2 == 0:
                            nc.scalar.mul(
                                out=hst[:, j, :], in_=ps[:],
                                mul=disn_s[:, nb : nb + 1],
                            )
                        else:
                            nc.vector.tensor_scalar(
                                hst[:, j, :],
                                ps[:],
                                disn_s[:, nb : nb + 1],
                                None,
                                mybir.AluOpType.mult,
                            )
                    nc.sync.dma_start(
                        out=hdst[nb0 * 128 : (nb0 + gs) * 128, :].rearrange(
                            "(j q) f -> q j f", q=128
                        ),
                        in_=hst[:, :gs, :],
                    )

            def edge_phase(layer):
                htab = h1_t if layer == 1 else h2_t
                bias_s = bias1_s if layer == 1 else bias2_s
                # bulk gathers of h'[src]. L and H regions rotate separate
                # buffer tags (a block consumes an early L call together with
                # a late H call; one shared tag deadlocks the rotation).
                call_tiles = []
                for gi, (is_h, coff, cn) in enumerate(p.calls):
                    gt = wpool.tile(
                        [128, GCMAX, 128],
                        BF16,
                        tag=("gbufH" if is_h else "gbufL"),
                        bufs=_GBUFS,
                        name="gt",
                    )
                    tab = htab[p.HALF :, :] if is_h else htab[: p.HALF, :]
                    ni = cn * 128
                    soff = coff * 128
                    nc.gpsimd.dma_gather(
                        gt[:, :cn, :],
                        tab,
                        idx_s[:, soff // 16 : (soff + ni) // 16],
                        ni,
                        ni,
                        128,
                        elem_step=128,
                        single_packet=(ni <= 1024),
                        queue_num=gi % _NQUEUES,
                    )
                    call_tiles.append(gt)

                if layer == 1:
                    a1st = wpool.tile(
                        [128, p.B * 128], BF16, tag="a1st", bufs=1, name="a1st"
                    )
                else:
                    outst = wpool.tile(
                        [128, p.B, 128], F32, tag="outst", bufs=1, name="outst"
                    )

                for b in range(p.B):
                    agg = pspool.tile([128, 128], F32, tag="agg", name="agg")
                    chunk_ids = [p.lofs[b] + c for c in range(p.chl[b])] + [
                        p.hofs[b] + c for c in range(p.chh[b])
                    ]
                    nch = len(chunk_ids)
                    for k, ci in enumerate(chunk_ids):
                        pt = wpool.tile(
                            [128, 128], BF16, tag="ptile", bufs=4, name="pt"
                        )
                        nc.vector.tensor_scalar(
                            pt[:],
                            iota_s[:],
                            dln_s[:, ci : ci + 1],
                            None,
                            mybir.AluOpType.is_equal,
                        )
                        gi, c = p.chunk_call[ci]
                        nc.tensor.matmul(
                            out=agg[:],
                            lhsT=pt[:],
                            rhs=call_tiles[gi][:, c : c + 1, :],
                            start=(k == 0),
                            stop=(k == nch - 1),
                        )
                    # epilogue: dst-side dis scale + bias (+ leaky relu)
                    t1 = wpool.tile([128, 128], F32, tag="ep1", name="t1")
                    nc.vector.tensor_scalar(
                        t1[:],
                        agg[:],
                        diso_s[:, b : b + 1],
                        None,
                        mybir.AluOpType.mult,
                    )
                    t2 = wpool.tile([128, 128], F32, tag="ep2", name="t2")
                    nc.vector.tensor_tensor(
                        out=t2[:], in0=t1[:], in1=bias_s[:], op=mybir.AluOpType.add
                    )
                    if layer == 1:
                        t3 = wpool.tile([128, 128], F32, tag="ep3", name="t3")
                        nc.vector.tensor_scalar(
                            t3[:], t2[:], NEG_SLOPE, None, mybir.AluOpType.mult
                        )
                        a1b = wpool.tile([128, 128], BF16, tag="a1b", name="a1b")
                        nc.vector.tensor_tensor(
                            out=a1b[:], in0=t2[:], in1=t3[:], op=mybir.AluOpType.max
                        )
                        tp = pspool.tile(
                            [128, 128], BF16, space="PSUM", tag="tp", name="tp"
                        )
                        nc.tensor.transpose(
                            out=tp[:], in_=a1b[:], identity=ident_s[:]
                        )
                        if b % 2 == 0:
                            nc.scalar.copy(
                                out=a1st[:, b * 128 : (b + 1) * 128], in_=tp[:]
                            )
                        else:
                            nc.vector.tensor_copy(
                                out=a1st[:, b * 128 : (b + 1) * 128], in_=tp[:]
                            )
                    else:
                        nc.vector.tensor_copy(out=outst[:, b, :], in_=t2[:])

                if layer == 1:
                    return a1st
                nc.sync.dma_start(
                    out=out_d.ap().rearrange("(b q) f -> q b f", q=128),
                    in_=outst[:, :, :],
                )
                return None

            def ag_phase(a1st):
                nc.sync.dma_start(out=agin_t[:, :], in_=a1st[:, :])
                nc.gpsimd.collective_compute(
                    "AllGather",
                    mybir.AluOpType.bypass,
                    replica_groups=[list(range(NCORES))],
                    ins=[agin_t[:, :].opt()],
                    outs=[agout_t[:, :].opt()],
                )

            h_phase(1)
            a1st = edge_phase(1)
            ag_phase(a1st)
            h_phase(2)
            edge_phase(2)

    nc.compile()
    return nc


_CACHE = {}


def _get_program(plan):
    nc = _CACHE.get(plan.key)
    if nc is None:
        nc = build_program(plan)
        _CACHE[plan.key] = nc
    return nc


def kernel(x, edge_index, batch, W1, b1, W2, b2):
    from concourse.bass_utils import run_bass_kernel_spmd

    x = np.asarray(x, np.float32)
    edge_index = np.asarray(edge_index)
    plan = make_plan(x.shape[0], edge_index)
    in_maps = make_in_maps(
        plan,
        x,
        np.asarray(W1, np.float32),
        np.asarray(b1, np.float32),
        np.asarray(W2, np.float32),
        np.asarray(b2, np.float32),
    )
    nc = _get_program(plan)
    res = run_bass_kernel_spmd(nc, in_maps, core_ids=list(range(NCORES)))
    out = np.concatenate([res.results[k]["out"] for k in range(NCORES)], axis=0)
    return np.ascontiguousarray(out[: plan.N]).astype(np.float32)


# revision 35
# speedup vs baseline: 1.8777x; 1.0030x over previous
"""2-layer GCN (GCNConv -> LeakyReLU -> GCNConv) on 8 Trainium2 NeuronCores.

Strategy: dst-partition the graph across 8 cores (each core owns N/8
destination rows and all edges pointing into them). Every core computes the
full dense h' = (x @ W.T) * dis[row] (replicated, cheap; dis = deg^-1/2 is
folded into the PSUM->SBUF copy), writes it row-major bf16 to local HBM,
bulk-gathers h'[src] for its edges with dma_gather (int16 indices; the node
table is split in two halves so indices fit in int16), and aggregates with
0/1 one-hot matmuls accumulated in PSUM; the dst-side dis[dst] scale and the
bias are applied per 128-row block in the epilogue. The symmetric norm
dis[src]*dis[dst] therefore never appears on the per-edge path. Self-loops
are materialized as explicit edges on the host. Between layers the per-core
activations are PE-transposed and AllGathered so layer 2 consumes them
directly as matmul lhsT.
"""

import math
import os as _os

import numpy as np
import ml_dtypes

from concourse import bacc, bass, mybir
import concourse.tile as tile

BF16 = mybir.dt.bfloat16
F32 = mybir.dt.float32
I16 = mybir.dt.int16

NCORES = 8
D = 128
NEG_SLOPE = 0.01
GCMAX = int(_os.environ.get("GCN_GCMAX", "8"))  # chunks per dma_gather call
_NQUEUES = int(_os.environ.get("GCN_NQUEUES", "4"))
_GBUFS = int(_os.environ.get("GCN_GBUFS", "16"))  # gather tile bufs per region
_PREP = bool(int(_os.environ.get("GCN_PREP", "0")))  # prep/trigger split
_PREPW = int(_os.environ.get("GCN_PREPW", "12"))  # prep lookahead window
HGROUP = 8  # h-compute blocks per DMA group


class Plan:
    pass


def make_plan(n_nodes, edge_index):
    """Host-side graph preprocessing: padding, degrees, self-loop edges,
    per-core dst-partitioned + per-(block,half) chunked edge slots."""
    p = Plan()
    src = edge_index[0].astype(np.int64)
    dst = edge_index[1].astype(np.int64)

    unit = NCORES * 128
    p.N = n_nodes
    p.NPAD = ((n_nodes + unit - 1) // unit) * unit
    p.PCN = p.NPAD // NCORES
    p.B = p.PCN // 128
    p.NB = p.NPAD // 128
    p.HALF = p.NPAD // 2
    assert p.HALF - 1 <= 32767, "node count too large for int16 half-split"

    deg = np.bincount(dst, minlength=p.NPAD).astype(np.float32) + 1.0
    dis = (1.0 / np.sqrt(deg)).astype(np.float32)
    p.dis = dis

    # self-loop edges: with the separable norm, a self edge at (i, i) with a
    # 0/1 one-hot contributes dis[i]*h[i]*dis[i] = the reference's analytic
    # self-loop term
    alln = np.arange(p.NPAD, dtype=np.int64)
    src_a = np.concatenate([src, alln])
    dst_a = np.concatenate([dst, alln])

    core = dst_a // p.PCN
    lb = (dst_a % p.PCN) // 128
    dloc = (dst_a % 128).astype(np.float32)
    halfbit = (src_a >= p.HALF).astype(np.int64)
    seg = (core * p.B + lb) * 2 + halfbit
    nseg = NCORES * p.B * 2

    order = np.lexsort((src_a, seg))
    seg_s = seg[order]
    src_s = src_a[order]
    dloc_s = dloc[order]

    counts = np.bincount(seg_s, minlength=nseg)
    cnt = counts.reshape(NCORES, p.B, 2)
    # per-(block,half) chunk counts, shared across cores (max over cores)
    p.chl = [max(1, int(math.ceil(cnt[:, b, 0].max() / 128))) for b in range(p.B)]
    p.chh = [max(1, int(math.ceil(cnt[:, b, 1].max() / 128))) for b in range(p.B)]
    p.SLch = sum(p.chl)
    p.SHch = sum(p.chh)
    p.NCH = p.SLch + p.SHch
    p.STOT = p.NCH * 128
    p.lofs = np.concatenate([[0], np.cumsum(p.chl)])[:-1]
    p.hofs = p.SLch + np.concatenate([[0], np.cumsum(p.chh)])[:-1]

    segid = np.arange(nseg)
    sblk = (segid // 2) % p.B
    sh = segid % 2
    base = np.where(sh == 0, p.lofs[sblk] * 128, p.hofs[sblk] * 128)

    seg_starts = np.zeros(nseg + 1, np.int64)
    np.cumsum(counts, out=seg_starts[1:])
    rank = np.arange(len(seg_s)) - seg_starts[seg_s]
    slot = base[seg_s] + rank
    corefor = seg_s // (2 * p.B)

    idx_all = np.zeros((NCORES, p.STOT), np.int32)
    # pad slots keep dst_local = -1 so is_equal(iota, -1) zeroes their column
    dl_all = np.full((NCORES, p.STOT), -1.0, np.float32)
    val = np.where(src_s >= p.HALF, src_s - p.HALF, src_s)
    idx_all[corefor, slot] = val
    dl_all[corefor, slot] = dloc_s

    # dma_gather index layout: [128, STOT/16] int16, slot s at [s%16, s//16],
    # replicated across the 8 groups of 16 partitions
    idx16 = idx_all.astype(np.int16).reshape(NCORES, p.STOT // 16, 16)
    idx16 = np.ascontiguousarray(idx16.transpose(0, 2, 1))
    p.idx16 = np.ascontiguousarray(np.tile(idx16, (1, 8, 1)))
    # per-chunk dst_local metadata, [128, NCH] with column = chunk
    p.dl = np.ascontiguousarray(dl_all.reshape(NCORES, p.NCH, 128).transpose(0, 2, 1))

    # per-node dis in device layouts
    p.disn = np.ascontiguousarray(
        dis.reshape(p.NB, 128).T
    )  # [128, NB], node nb*128+q at [q, nb]
    p.diso = np.ascontiguousarray(
        dis.reshape(NCORES, p.B, 128).transpose(0, 2, 1)
    )  # [C, 128, B]

    # gather call plan: (is_h, chunk_off_in_global_chunkspace, nchunks)
    p.calls = []
    for is_h, n_region, off in ((0, p.SLch, 0), (1, p.SHch, p.SLch)):
        nc_calls = max(1, math.ceil(n_region / GCMAX))
        per = math.ceil(n_region / nc_calls)
        c0 = 0
        while c0 < n_region:
            cn = min(per, n_region - c0)
            p.calls.append((is_h, off + c0, cn))
            c0 += cn
    p.chunk_call = np.zeros((p.NCH, 2), np.int64)
    for gi, (_, coff, cn) in enumerate(p.calls):
        for c in range(cn):
            p.chunk_call[coff + c] = (gi, c)

    p.key = (p.NPAD, p.B, tuple(p.chl), tuple(p.chh))
    return p


def make_in_maps(plan, x, W1, b1, W2, b2):
    p = plan
    xpad = np.zeros((p.NPAD, D), np.float32)
    xpad[: p.N] = x
    xT = np.ascontiguousarray(xpad.T).astype(ml_dtypes.bfloat16)

    iota = np.tile(np.arange(128, dtype=np.float32)[None, :], (128, 1))
    ident = np.eye(128, dtype=np.float32)

    common = {
        "xT": xT,
        "w1t": np.ascontiguousarray(W1.T).astype(ml_dtypes.bfloat16),
        "w2t": np.ascontiguousarray(W2.T).astype(ml_dtypes.bfloat16),
        "iota": iota.astype(ml_dtypes.bfloat16),
        "ident": ident.astype(ml_dtypes.bfloat16),
        "bias1": np.ascontiguousarray(
            np.tile(np.asarray(b1, np.float32)[None, :], (128, 1))
        ),
        "bias2": np.ascontiguousarray(
            np.tile(np.asarray(b2, np.float32)[None, :], (128, 1))
        ),
        "disn": p.disn,
    }
    return [
        dict(common, idx=p.idx16[k], dln=p.dl[k], diso=p.diso[k])
        for k in range(NCORES)
    ]


def build_program(plan):
    p = plan

    nc = bacc.Bacc(
        "TRN2",
        target_bir_lowering=False,
        debug=False,
        num_devices=NCORES,
        num_swdge_queues=_NQUEUES,
    )

    xT_d = nc.dram_tensor("xT", [128, p.NPAD], BF16, kind="ExternalInput")
    w1t_d = nc.dram_tensor("w1t", [128, 128], BF16, kind="ExternalInput")
    w2t_d = nc.dram_tensor("w2t", [128, 128], BF16, kind="ExternalInput")
    iota_d = nc.dram_tensor("iota", [128, 128], BF16, kind="ExternalInput")
    ident_d = nc.dram_tensor("ident", [128, 128], BF16, kind="ExternalInput")
    bias1_d = nc.dram_tensor("bias1", [128, 128], F32, kind="ExternalInput")
    bias2_d = nc.dram_tensor("bias2", [128, 128], F32, kind="ExternalInput")
    disn_d = nc.dram_tensor("disn", [128, p.NB], F32, kind="ExternalInput")
    diso_d = nc.dram_tensor("diso", [128, p.B], F32, kind="ExternalInput")
    idx_d = nc.dram_tensor("idx", [128, p.STOT // 16], I16, kind="ExternalInput")
    dln_d = nc.dram_tensor("dln", [128, p.NCH], F32, kind="ExternalInput")
    out_d = nc.dram_tensor("out", [p.PCN, 128], F32, kind="ExternalOutput")

    with tile.TileContext(nc) as tc:
        with (
            tc.tile_pool(name="dram", bufs=1, space="DRAM") as dpool,
            tc.tile_pool(name="const", bufs=1) as cpool,
            tc.tile_pool(name="work", bufs=2) as wpool,
            tc.tile_pool(name="psum", bufs=2, space="PSUM") as pspool,
        ):
            h1_t = dpool.tile([p.NPAD, 128], BF16, name="h1buf")
            h2_t = dpool.tile([p.NPAD, 128], BF16, name="h2buf")
            agin_t = dpool.tile([128, p.PCN], BF16, name="aginbuf")
            # (addr_space="Shared" would be faster for the collective, but
            # neuronxcc's DataLocalityOpt crashes on DMA loads from Shared
            # scratchpad tensors)
            agout_t = dpool.tile([NCORES * 128, p.PCN], BF16, name="agoutbuf")

            def cload(dram, shape, dtype, name):
                t = cpool.tile(shape, dtype, name=name)
                nc.sync.dma_start(out=t[:], in_=dram.ap())
                return t

            w1t_s = cload(w1t_d, [128, 128], BF16, "w1t_s")
            w2t_s = cload(w2t_d, [128, 128], BF16, "w2t_s")
            iota_s = cload(iota_d, [128, 128], BF16, "iota_s")
            ident_s = cload(ident_d, [128, 128], BF16, "ident_s")
            bias1_s = cload(bias1_d, [128, 128], F32, "bias1_s")
            bias2_s = cload(bias2_d, [128, 128], F32, "bias2_s")
            disn_s = cload(disn_d, [128, p.NB], F32, "disn_s")
            diso_s = cload(diso_d, [128, p.B], F32, "diso_s")
            idx_s = cload(idx_d, [128, p.STOT // 16], I16, "idx_s")
            dln_s = cload(dln_d, [128, p.NCH], F32, "dln_s")

            # --- prep/trigger pipelined gathers (both layers unified) ---
            # Descriptor generation (Q7 software, the serial bottleneck) has
            # no dependency on the h tables, so prepare_only preps run far
            # ahead — layer 2's generation overlaps layer 1's compute. The
            # trigger carries the h-table read dependency instead.
            all_calls = [
                (layer, is_h, coff, cn)
                for layer in (1, 2)
                for (is_h, coff, cn) in p.calls
            ]
            qsems = (
                [nc.alloc_semaphore(f"gsem{q}") for q in range(_NQUEUES)]
                if _PREP
                else None
            )
            gstate = {"prep": 0, "trig": 0, "tiles": {}}

            def emit_prep(k):
                layer, is_h, coff, cn = all_calls[k]
                htab_k = h1_t if layer == 1 else h2_t
                gt = wpool.tile(
                    [128, GCMAX, 128],
                    BF16,
                    tag=("gbufH" if is_h else "gbufL"),
                    bufs=_GBUFS,
                    name="gt",
                )
                tab = htab_k[p.HALF :, :] if is_h else htab_k[: p.HALF, :]
                ni = cn * 128
                soff = coff * 128
                q = k % _NQUEUES
                nc.gpsimd.dma_gather(
                    gt[:, :cn, :],
                    tab,
                    idx_s[:, soff // 16 : (soff + ni) // 16],
                    ni,
                    ni,
                    128,
                    elem_step=128,
                    single_packet=(ni <= 1024),
                    queue_num=q,
                    prepare_only=True,
                    sem=qsems[q],
                )
                gstate["tiles"][k] = gt

            # Tile only wires consumer waits for trigger_dma(count=None),
            # which fires a queue's whole pending set — so a pending set must
            # never mix layers (an L2 prep's h2 dependency on a trigger that
            # L1 consumers wait on would deadlock). Prep emission is capped at
            # the layer boundary until that layer is flushed.
            def emit_preps_until(limit, cap):
                while gstate["prep"] < min(cap, len(all_calls)):
                    if gstate["prep"] >= limit:
                        break
                    emit_prep(gstate["prep"])
                    gstate["prep"] += 1

            def _fire_all_pending():
                for q in range(_NQUEUES):
                    if any(
                        t % _NQUEUES == q
                        for t in range(gstate["trig"], gstate["prep"])
                    ):
                        nc.gpsimd.trigger_dma(count=None, queue_num=q)
                gstate["trig"] = gstate["prep"]

            def ensure_triggered(k, layer_cap):
                if k >= gstate["trig"]:
                    _fire_all_pending()
                    emit_preps_until(layer_cap, gstate["trig"] + _PREPW)

            def flush_layer(layer_cap):
                _fire_all_pending()
                emit_preps_until(layer_cap, len(all_calls))

            if _PREP:
                emit_preps_until(len(p.calls), _PREPW)

            def h_phase(layer):
                """h' = (lhs @ W.T) * dis[row] for all NPAD nodes -> DRAM."""
                wst = w1t_s if layer == 1 else w2t_s
                hdst = h1_t if layer == 1 else h2_t
                groups = []
                if layer == 1:
                    nb0 = 0
                    while nb0 < p.NB:
                        gs = min(HGROUP, p.NB - nb0)
                        groups.append((nb0, gs))
                        nb0 += gs
                else:
                    # lhs tiles come from the AllGather output; groups must
                    # not cross rank boundaries
                    for r in range(NCORES):
                        lb0 = 0
                        while lb0 < p.B:
                            gs = min(HGROUP, p.B - lb0)
                            groups.append((r * p.B + lb0, gs))
                            lb0 += gs
                for nb0, gs in groups:
                    lhs_g = wpool.tile(
                        [128, HGROUP * 128], BF16, tag="lhsg", name="lhs_g"
                    )
                    if layer == 1:
                        nc.sync.dma_start(
                            out=lhs_g[:, : gs * 128],
                            in_=xT_d[:, nb0 * 128 : (nb0 + gs) * 128],
                        )
                    else:
                        r = nb0 // p.B
                        lb0 = nb0 - r * p.B
                        nc.sync.dma_start(
                            out=lhs_g[:, : gs * 128],
                            in_=agout_t[
                                r * 128 : (r + 1) * 128,
                                lb0 * 128 : (lb0 + gs) * 128,
                            ],
                        )
                    hst = wpool.tile(
                        [128, HGROUP, 128], BF16, tag="hst", bufs=3, name="hst"
                    )
                    for j in range(gs):
                        nb = nb0 + j
                        ps = pspool.tile(
                            [128, 128], F32, tag="hps", bufs=4, name="hps"
                        )
                        nc.tensor.matmul(
                            out=ps[:],
                            lhsT=lhs_g[:, j * 128 : (j + 1) * 128],
                            rhs=wst[:],
                            start=True,
                            stop=True,
                        )
                        # PSUM -> SBUF copy doubles as the dis[row] scale
                        if j % 2 == 0:
                            nc.scalar.mul(
                                out=hst[:, j, :], in_=ps[:],
                                mul=disn_s[:, nb : nb + 1],
                            )
                        else:
                            nc.vector.tensor_scalar(
                                hst[:, j, :],
                                ps[:],
                                disn_s[:, nb : nb + 1],
                                None,
                                mybir.AluOpType.mult,
                            )
                    nc.sync.dma_start(
                        out=hdst[nb0 * 128 : (nb0 + gs) * 128, :].rearrange(
                            "(j q) f -> q j f", q=128
                        ),
                        in_=hst[:, :gs, :],
                    )

            def edge_phase(layer):
                htab = h1_t if layer == 1 else h2_t
                bias_s = bias1_s if layer == 1 else bias2_s
                # bulk gathers of h'[src]. L and H regions rotate separate
                # buffer tags (a block consumes an early L call together with
                # a late H call; one shared tag deadlocks the rotation).
                cbase = 0 if layer == 1 else len(p.calls)
                call_tiles = []
                if not _PREP:
                    for gi, (is_h, coff, cn) in enumerate(p.calls):
                        gt = wpool.tile(
                            [128, GCMAX, 128],
                            BF16,
                            tag=("gbufH" if is_h else "gbufL"),
                            bufs=_GBUFS,
                            name="gt",
                        )
                        tab = htab[p.HALF :, :] if is_h else htab[: p.HALF, :]
                        ni = cn * 128
                        soff = coff * 128
                        nc.gpsimd.dma_gather(
                            gt[:, :cn, :],
                            tab,
                            idx_s[:, soff // 16 : (soff + ni) // 16],
                            ni,
                            ni,
                            128,
                            elem_step=128,
                            single_packet=(ni <= 1024),
                            queue_num=gi % _NQUEUES,
                        )
                        call_tiles.append(gt)

                if layer == 1:
                    a1st = wpool.tile(
                        [128, p.B * 128], BF16, tag="a1st", bufs=1, name="a1st"
                    )
                else:
                    outst = wpool.tile(
                        [128, p.B, 128], F32, tag="outst", bufs=1, name="outst"
                    )

                for b in range(p.B):
                    agg = pspool.tile([128, 128], F32, tag="agg", name="agg")
                    chunk_ids = [p.lofs[b] + c for c in range(p.chl[b])] + [
                        p.hofs[b] + c for c in range(p.chh[b])
                    ]
                    nch = len(chunk_ids)
                    if _PREP:
                        ensure_triggered(
                            cbase + max(p.chunk_call[ci][0] for ci in chunk_ids),
                            (layer - 1) * len(p.calls) + len(p.calls),
                        )
                    for k, ci in enumerate(chunk_ids):
                        pt = wpool.tile(
                            [128, 128], BF16, tag="ptile", bufs=4, name="pt"
                        )
                        nc.vector.tensor_scalar(
                            pt[:],
                            iota_s[:],
                            dln_s[:, ci : ci + 1],
                            None,
                            mybir.AluOpType.is_equal,
                        )
                        gi, c = p.chunk_call[ci]
                        gtile = (
                            gstate["tiles"][cbase + gi] if _PREP else call_tiles[gi]
                        )
                        nc.tensor.matmul(
                            out=agg[:],
                            lhsT=pt[:],
                            rhs=gtile[:, c : c + 1, :],
                            start=(k == 0),
                            stop=(k == nch - 1),
                        )
                    # epilogue: dst-side dis scale + bias (+ leaky relu)
                    t1 = wpool.tile([128, 128], F32, tag="ep1", name="t1")
                    nc.vector.tensor_scalar(
                        t1[:],
                        agg[:],
                        diso_s[:, b : b + 1],
                        None,
                        mybir.AluOpType.mult,
                    )
                    t2 = wpool.tile([128, 128], F32, tag="ep2", name="t2")
                    nc.vector.tensor_tensor(
                        out=t2[:], in0=t1[:], in1=bias_s[:], op=mybir.AluOpType.add
                    )
                    if layer == 1:
                        t3 = wpool.tile([128, 128], F32, tag="ep3", name="t3")
                        nc.vector.tensor_scalar(
                            t3[:], t2[:], NEG_SLOPE, None, mybir.AluOpType.mult
                        )
                        a1b = wpool.tile([128, 128], BF16, tag="a1b", name="a1b")
                        nc.vector.tensor_tensor(
                            out=a1b[:], in0=t2[:], in1=t3[:], op=mybir.AluOpType.max
                        )
                        tp = pspool.tile(
                            [128, 128], BF16, space="PSUM", tag="tp", name="tp"
                        )
                        nc.tensor.transpose(
                            out=tp[:], in_=a1b[:], identity=ident_s[:]
                        )
                        if b % 2 == 0:
                            nc.scalar.copy(
                                out=a1st[:, b * 128 : (b + 1) * 128], in_=tp[:]
                            )
                        else:
                            nc.vector.tensor_copy(
                                out=a1st[:, b * 128 : (b + 1) * 128], in_=tp[:]
                            )
                    else:
                        nc.vector.tensor_copy(out=outst[:, b, :], in_=t2[:])

                if _PREP:
                    flush_layer(len(all_calls))
                if layer == 1:
                    return a1st
                nc.sync.dma_start(
                    out=out_d.ap().rearrange("(b q) f -> q b f", q=128),
                    in_=outst[:, :, :],
                )
                return None

            def ag_phase(a1st):
                nc.sync.dma_start(out=agin_t[:, :], in_=a1st[:, :])
                nc.gpsimd.collective_compute(
                    "AllGather",
                    mybir.AluOpType.bypass,
                    replica_groups=[list(range(NCORES))],
                    ins=[agin_t[:, :].opt()],
                    outs=[agout_t[:, :].opt()],
                )

            h_phase(1)
            a1st = edge_phase(1)
            ag_phase(a1st)
            h_phase(2)
            edge_phase(2)

    nc.compile()
    return nc


_CACHE = {}


def _get_program(plan):
    nc = _CACHE.get(plan.key)
    if nc is None:
        nc = build_program(plan)
        _CACHE[plan.key] = nc
    return nc


def kernel(x, edge_index, batch, W1, b1, W2, b2):
    from concourse.bass_utils import run_bass_kernel_spmd

    x = np.asarray(x, np.float32)
    edge_index = np.asarray(edge_index)
    plan = make_plan(x.shape[0], edge_index)
    in_maps = make_in_maps(
        plan,
        x,
        np.asarray(W1, np.float32),
        np.asarray(b1, np.float32),
        np.asarray(W2, np.float32),
        np.asarray(b2, np.float32),
    )
    nc = _get_program(plan)
    res = run_bass_kernel_spmd(nc, in_maps, core_ids=list(range(NCORES)))
    out = np.concatenate([res.results[k]["out"] for k in range(NCORES)], axis=0)
    return np.ascontiguousarray(out[: plan.N]).astype(np.float32)


# revision 41
# speedup vs baseline: 1.9412x; 1.0339x over previous
"""2-layer GCN (GCNConv -> LeakyReLU -> GCNConv) on 8 Trainium2 NeuronCores.

Strategy: dst-partition the graph across 8 cores (each core owns N/8
destination rows and all edges pointing into them). Every core computes the
full dense h' = (x @ W.T) * dis[row] (replicated, cheap; dis = deg^-1/2 is
folded into the PSUM->SBUF copy), writes it row-major bf16 to local HBM,
bulk-gathers h'[src] for its edges with dma_gather (int16 indices; the node
table is split in two halves so indices fit in int16), and aggregates with
0/1 one-hot matmuls accumulated in PSUM; the dst-side dis[dst] scale and the
bias are applied per 128-row block in the epilogue. The symmetric norm
dis[src]*dis[dst] therefore never appears on the per-edge path. Self-loops
are materialized as explicit edges on the host. Between layers the per-core
activations are PE-transposed and AllGathered so layer 2 consumes them
directly as matmul lhsT.
"""

import math
import os as _os

import numpy as np
import ml_dtypes

from concourse import bacc, bass, mybir
import concourse.tile as tile

BF16 = mybir.dt.bfloat16
F32 = mybir.dt.float32
I16 = mybir.dt.int16

NCORES = 8
D = 128
NEG_SLOPE = 0.01
GCMAX = int(_os.environ.get("GCN_GCMAX", "8"))  # chunks per dma_gather call
_NQUEUES = int(_os.environ.get("GCN_NQUEUES", "4"))
_GBUFS = int(_os.environ.get("GCN_GBUFS", "16"))  # gather tile bufs per region
_PREP = bool(int(_os.environ.get("GCN_PREP", "0")))  # prep/trigger split
_PREPW = int(_os.environ.get("GCN_PREPW", "12"))  # prep lookahead window
HGROUP = 8  # h-compute blocks per DMA group


class Plan:
    pass


def make_plan(n_nodes, edge_index):
    """Host-side graph preprocessing: padding, degrees, self-loop edges,
    per-core dst-partitioned + per-(block,half) chunked edge slots."""
    p = Plan()
    src = edge_index[0].astype(np.int64)
    dst = edge_index[1].astype(np.int64)

    unit = NCORES * 128
    p.N = n_nodes
    p.NPAD = ((n_nodes + unit - 1) // unit) * unit
    p.PCN = p.NPAD // NCORES
    p.B = p.PCN // 128
    p.NB = p.NPAD // 128
    p.HALF = p.NPAD // 2
    assert p.HALF - 1 <= 32767, "node count too large for int16 half-split"

    deg = np.bincount(dst, minlength=p.NPAD).astype(np.float32) + 1.0
    dis = (1.0 / np.sqrt(deg)).astype(np.float32)
    p.dis = dis

    # self-loop edges: with the separable norm, a self edge at (i, i) with a
    # 0/1 one-hot contributes dis[i]*h[i]*dis[i] = the reference's analytic
    # self-loop term
    alln = np.arange(p.NPAD, dtype=np.int64)
    src_a = np.concatenate([src, alln])
    dst_a = np.concatenate([dst, alln])

    core = dst_a // p.PCN
    lb = (dst_a % p.PCN) // 128
    dloc = (dst_a % 128).astype(np.float32)
    halfbit = (src_a >= p.HALF).astype(np.int64)
    seg = (core * p.B + lb) * 2 + halfbit
    nseg = NCORES * p.B * 2

    order = np.lexsort((src_a, seg))
    seg_s = seg[order]
    src_s = src_a[order]
    dloc_s = dloc[order]

    counts = np.bincount(seg_s, minlength=nseg)
    cnt = counts.reshape(NCORES, p.B, 2)
    # per-(block,half) chunk counts, shared across cores (max over cores)
    p.chl = [max(1, int(math.ceil(cnt[:, b, 0].max() / 128))) for b in range(p.B)]
    p.chh = [max(1, int(math.ceil(cnt[:, b, 1].max() / 128))) for b in range(p.B)]
    p.SLch = sum(p.chl)
    p.SHch = sum(p.chh)
    p.NCH = p.SLch + p.SHch
    p.STOT = p.NCH * 128
    p.lofs = np.concatenate([[0], np.cumsum(p.chl)])[:-1]
    p.hofs = p.SLch + np.concatenate([[0], np.cumsum(p.chh)])[:-1]

    segid = np.arange(nseg)
    sblk = (segid // 2) % p.B
    sh = segid % 2
    base = np.where(sh == 0, p.lofs[sblk] * 128, p.hofs[sblk] * 128)

    seg_starts = np.zeros(nseg + 1, np.int64)
    np.cumsum(counts, out=seg_starts[1:])
    rank = np.arange(len(seg_s)) - seg_starts[seg_s]
    slot = base[seg_s] + rank
    corefor = seg_s // (2 * p.B)

    idx_all = np.zeros((NCORES, p.STOT), np.int32)
    # pad slots keep dst_local = -1 so is_equal(iota, -1) zeroes their column
    dl_all = np.full((NCORES, p.STOT), -1.0, np.float32)
    val = np.where(src_s >= p.HALF, src_s - p.HALF, src_s)
    idx_all[corefor, slot] = val
    dl_all[corefor, slot] = dloc_s

    # dma_gather index layout: [128, STOT/16] int16, slot s at [s%16, s//16],
    # replicated across the 8 groups of 16 partitions
    idx16 = idx_all.astype(np.int16).reshape(NCORES, p.STOT // 16, 16)
    idx16 = np.ascontiguousarray(idx16.transpose(0, 2, 1))
    p.idx16 = np.ascontiguousarray(np.tile(idx16, (1, 8, 1)))
    # per-chunk dst_local metadata, [128, NCH] with column = chunk
    p.dl = np.ascontiguousarray(dl_all.reshape(NCORES, p.NCH, 128).transpose(0, 2, 1))

    # per-node dis in device layouts
    p.disn = np.ascontiguousarray(
        dis.reshape(p.NB, 128).T
    )  # [128, NB], node nb*128+q at [q, nb]
    p.diso = np.ascontiguousarray(
        dis.reshape(NCORES, p.B, 128).transpose(0, 2, 1)
    )  # [C, 128, B]

    # gather call plan: (is_h, chunk_off_in_global_chunkspace, nchunks)
    p.calls = []
    for is_h, n_region, off in ((0, p.SLch, 0), (1, p.SHch, p.SLch)):
        nc_calls = max(1, math.ceil(n_region / GCMAX))
        per = math.ceil(n_region / nc_calls)
        c0 = 0
        while c0 < n_region:
            cn = min(per, n_region - c0)
            p.calls.append((is_h, off + c0, cn))
            c0 += cn
    p.chunk_call = np.zeros((p.NCH, 2), np.int64)
    for gi, (_, coff, cn) in enumerate(p.calls):
        for c in range(cn):
            p.chunk_call[coff + c] = (gi, c)

    p.key = (p.NPAD, p.B, tuple(p.chl), tuple(p.chh))
    return p


def make_in_maps(plan, x, W1, b1, W2, b2):
    p = plan
    xpad = np.zeros((p.NPAD, D), np.float32)
    xpad[: p.N] = x
    xT = np.ascontiguousarray(xpad.T).astype(ml_dtypes.bfloat16)

    iota = np.tile(np.arange(128, dtype=np.float32)[None, :], (128, 1))
    ident = np.eye(128, dtype=np.float32)

    common = {
        "xT": xT,
        "w1t": np.ascontiguousarray(W1.T).astype(ml_dtypes.bfloat16),
        "w2t": np.ascontiguousarray(W2.T).astype(ml_dtypes.bfloat16),
        "iota": iota.astype(ml_dtypes.bfloat16),
        "ident": ident.astype(ml_dtypes.bfloat16),
        "bias1": np.ascontiguousarray(
            np.tile(np.asarray(b1, np.float32)[None, :], (128, 1))
        ),
        "bias2": np.ascontiguousarray(
            np.tile(np.asarray(b2, np.float32)[None, :], (128, 1))
        ),
        "disn": p.disn,
    }
    return [
        dict(common, idx=p.idx16[k], dln=p.dl[k], diso=p.diso[k])
        for k in range(NCORES)
    ]


def build_program(plan):
    p = plan

    nc = bacc.Bacc(
        "TRN2",
        target_bir_lowering=False,
        debug=False,
        num_devices=NCORES,
        num_swdge_queues=_NQUEUES,
    )

    xT_d = nc.dram_tensor("xT", [128, p.NPAD], BF16, kind="ExternalInput")
    w1t_d = nc.dram_tensor("w1t", [128, 128], BF16, kind="ExternalInput")
    w2t_d = nc.dram_tensor("w2t", [128, 128], BF16, kind="ExternalInput")
    iota_d = nc.dram_tensor("iota", [128, 128], BF16, kind="ExternalInput")
    ident_d = nc.dram_tensor("ident", [128, 128], BF16, kind="ExternalInput")
    bias1_d = nc.dram_tensor("bias1", [128, 128], F32, kind="ExternalInput")
    bias2_d = nc.dram_tensor("bias2", [128, 128], F32, kind="ExternalInput")
    disn_d = nc.dram_tensor("disn", [128, p.NB], F32, kind="ExternalInput")
    diso_d = nc.dram_tensor("diso", [128, p.B], F32, kind="ExternalInput")
    idx_d = nc.dram_tensor("idx", [128, p.STOT // 16], I16, kind="ExternalInput")
    dln_d = nc.dram_tensor("dln", [128, p.NCH], F32, kind="ExternalInput")
    out_d = nc.dram_tensor("out", [p.PCN, 128], F32, kind="ExternalOutput")

    with tile.TileContext(nc) as tc:
        with (
            tc.tile_pool(name="dram", bufs=1, space="DRAM") as dpool,
            tc.tile_pool(name="const", bufs=1) as cpool,
            tc.tile_pool(name="work", bufs=2) as wpool,
            tc.tile_pool(name="psum", bufs=2, space="PSUM") as pspool,
        ):
            # each h table is split into lo/hi halves as separate tensors so
            # the L-region gathers depend only on the lower half's writes
            # (whole-tensor deps would stall gathers behind the full h phase)
            h1lo_t = dpool.tile([p.HALF, 128], BF16, name="h1lo")
            h1hi_t = dpool.tile([p.HALF, 128], BF16, name="h1hi")
            h2lo_t = dpool.tile([p.HALF, 128], BF16, name="h2lo")
            h2hi_t = dpool.tile([p.HALF, 128], BF16, name="h2hi")
            agin_t = dpool.tile([128, p.PCN], BF16, name="aginbuf")
            # (addr_space="Shared" would be faster for the collective, but
            # neuronxcc's DataLocalityOpt crashes on DMA loads from Shared
            # scratchpad tensors)
            agout_t = dpool.tile([NCORES * 128, p.PCN], BF16, name="agoutbuf")

            def cload(dram, shape, dtype, name):
                t = cpool.tile(shape, dtype, name=name)
                nc.sync.dma_start(out=t[:], in_=dram.ap())
                return t

            w1t_s = cload(w1t_d, [128, 128], BF16, "w1t_s")
            w2t_s = cload(w2t_d, [128, 128], BF16, "w2t_s")
            iota_s = cload(iota_d, [128, 128], BF16, "iota_s")
            ident_s = cload(ident_d, [128, 128], BF16, "ident_s")
            bias1_s = cload(bias1_d, [128, 128], F32, "bias1_s")
            bias2_s = cload(bias2_d, [128, 128], F32, "bias2_s")
            disn_s = cload(disn_d, [128, p.NB], F32, "disn_s")
            diso_s = cload(diso_d, [128, p.B], F32, "diso_s")
            idx_s = cload(idx_d, [128, p.STOT // 16], I16, "idx_s")
            dln_s = cload(dln_d, [128, p.NCH], F32, "dln_s")

            # --- prep/trigger pipelined gathers (both layers unified) ---
            # Descriptor generation (Q7 software, the serial bottleneck) has
            # no dependency on the h tables, so prepare_only preps run far
            # ahead — layer 2's generation overlaps layer 1's compute. The
            # trigger carries the h-table read dependency instead.
            all_calls = [
                (layer, is_h, coff, cn)
                for layer in (1, 2)
                for (is_h, coff, cn) in p.calls
            ]
            qsems = (
                [nc.alloc_semaphore(f"gsem{q}") for q in range(_NQUEUES)]
                if _PREP
                else None
            )
            gstate = {"prep": 0, "trig": 0, "tiles": {}}

            def emit_prep(k):
                layer, is_h, coff, cn = all_calls[k]
                htab_k = None
                gt = wpool.tile(
                    [128, GCMAX, 128],
                    BF16,
                    tag=("gbufH" if is_h else "gbufL"),
                    bufs=_GBUFS,
                    name="gt",
                )
                tab = (
                    (h1hi_t if layer == 1 else h2hi_t)
                    if is_h
                    else (h1lo_t if layer == 1 else h2lo_t)
                )[:, :]
                ni = cn * 128
                soff = coff * 128
                q = k % _NQUEUES
                nc.gpsimd.dma_gather(
                    gt[:, :cn, :],
                    tab,
                    idx_s[:, soff // 16 : (soff + ni) // 16],
                    ni,
                    ni,
                    128,
                    elem_step=128,
                    single_packet=(ni <= 1024),
                    queue_num=q,
                    prepare_only=True,
                    sem=qsems[q],
                )
                gstate["tiles"][k] = gt

            # Tile only wires consumer waits for trigger_dma(count=None),
            # which fires a queue's whole pending set — so a pending set must
            # never mix layers (an L2 prep's h2 dependency on a trigger that
            # L1 consumers wait on would deadlock). Prep emission is capped at
            # the layer boundary until that layer is flushed.
            def emit_preps_until(limit, cap):
                while gstate["prep"] < min(cap, len(all_calls)):
                    if gstate["prep"] >= limit:
                        break
                    emit_prep(gstate["prep"])
                    gstate["prep"] += 1

            def _fire_all_pending():
                for q in range(_NQUEUES):
                    if any(
                        t % _NQUEUES == q
                        for t in range(gstate["trig"], gstate["prep"])
                    ):
                        nc.gpsimd.trigger_dma(count=None, queue_num=q)
                gstate["trig"] = gstate["prep"]

            def ensure_triggered(k, layer_cap):
                if k >= gstate["trig"]:
                    _fire_all_pending()
                    emit_preps_until(layer_cap, gstate["trig"] + _PREPW)

            def flush_layer(layer_cap):
                _fire_all_pending()
                emit_preps_until(layer_cap, len(all_calls))

            if _PREP:
                emit_preps_until(len(p.calls), _PREPW)

            def h_phase(layer):
                """h' = (lhs @ W.T) * dis[row] for all NPAD nodes -> DRAM."""
                wst = w1t_s if layer == 1 else w2t_s
                hlo = h1lo_t if layer == 1 else h2lo_t
                hhi = h1hi_t if layer == 1 else h2hi_t
                nbh = p.NB // 2  # first block of the upper half
                groups = []
                if layer == 1:
                    nb0 = 0
                    while nb0 < p.NB:
                        gs = min(HGROUP, p.NB - nb0)
                        if nb0 < nbh < nb0 + gs:
                            gs = nbh - nb0  # don't straddle the half boundary
                        groups.append((nb0, gs))
                        nb0 += gs
                else:
                    # lhs tiles come from the AllGather output; groups must
                    # not cross rank boundaries
                    for r in range(NCORES):
                        lb0 = 0
                        while lb0 < p.B:
                            gs = min(HGROUP, p.B - lb0)
                            groups.append((r * p.B + lb0, gs))
                            lb0 += gs
                for nb0, gs in groups:
                    lhs_g = wpool.tile(
                        [128, HGROUP * 128], BF16, tag="lhsg", name="lhs_g"
                    )
                    if layer == 1:
                        nc.sync.dma_start(
                            out=lhs_g[:, : gs * 128],
                            in_=xT_d[:, nb0 * 128 : (nb0 + gs) * 128],
                        )
                    else:
                        r = nb0 // p.B
                        lb0 = nb0 - r * p.B
                        nc.sync.dma_start(
                            out=lhs_g[:, : gs * 128],
                            in_=agout_t[
                                r * 128 : (r + 1) * 128,
                                lb0 * 128 : (lb0 + gs) * 128,
                            ],
                        )
                    hst = wpool.tile(
                        [128, HGROUP, 128], BF16, tag="hst", bufs=3, name="hst"
                    )
                    for j in range(gs):
                        nb = nb0 + j
                        ps = pspool.tile(
                            [128, 128], F32, tag="hps", bufs=4, name="hps"
                        )
                        nc.tensor.matmul(
                            out=ps[:],
                            lhsT=lhs_g[:, j * 128 : (j + 1) * 128],
                            rhs=wst[:],
                            start=True,
                            stop=True,
                        )
                        # PSUM -> SBUF copy doubles as the dis[row] scale
                        if j % 2 == 0:
                            nc.scalar.mul(
                                out=hst[:, j, :], in_=ps[:],
                                mul=disn_s[:, nb : nb + 1],
                            )
                        else:
                            nc.vector.tensor_scalar(
                                hst[:, j, :],
                                ps[:],
                                disn_s[:, nb : nb + 1],
                                None,
                                mybir.AluOpType.mult,
                            )
                    if nb0 >= nbh:
                        hw_dst, row0 = hhi, (nb0 - nbh) * 128
                    else:
                        hw_dst, row0 = hlo, nb0 * 128
                    nc.sync.dma_start(
                        out=hw_dst[row0 : row0 + gs * 128, :].rearrange(
                            "(j q) f -> q j f", q=128
                        ),
                        in_=hst[:, :gs, :],
                    )

            def edge_phase(layer):
                bias_s = bias1_s if layer == 1 else bias2_s
                # bulk gathers of h'[src]. L and H regions rotate separate
                # buffer tags (a block consumes an early L call together with
                # a late H call; one shared tag deadlocks the rotation).
                cbase = 0 if layer == 1 else len(p.calls)
                call_tiles = []
                if not _PREP:
                    for gi, (is_h, coff, cn) in enumerate(p.calls):
                        gt = wpool.tile(
                            [128, GCMAX, 128],
                            BF16,
                            tag=("gbufH" if is_h else "gbufL"),
                            bufs=_GBUFS,
                            name="gt",
                        )
                        tab = (
                            (h1hi_t if layer == 1 else h2hi_t)
                            if is_h
                            else (h1lo_t if layer == 1 else h2lo_t)
                        )[:, :]
                        ni = cn * 128
                        soff = coff * 128
                        nc.gpsimd.dma_gather(
                            gt[:, :cn, :],
                            tab,
                            idx_s[:, soff // 16 : (soff + ni) // 16],
                            ni,
                            ni,
                            128,
                            elem_step=128,
                            single_packet=(ni <= 1024),
                            queue_num=gi % _NQUEUES,
                        )
                        call_tiles.append(gt)

                if layer == 1:
                    a1st = wpool.tile(
                        [128, p.B * 128], BF16, tag="a1st", bufs=1, name="a1st"
                    )
                else:
                    outst = wpool.tile(
                        [128, p.B, 128], F32, tag="outst", bufs=1, name="outst"
                    )

                for b in range(p.B):
                    agg = pspool.tile([128, 128], F32, tag="agg", name="agg")
                    chunk_ids = [p.lofs[b] + c for c in range(p.chl[b])] + [
                        p.hofs[b] + c for c in range(p.chh[b])
                    ]
                    nch = len(chunk_ids)
                    if _PREP:
                        ensure_triggered(
                            cbase + max(p.chunk_call[ci][0] for ci in chunk_ids),
                            (layer - 1) * len(p.calls) + len(p.calls),
                        )
                    for k, ci in enumerate(chunk_ids):
                        pt = wpool.tile(
                            [128, 128], BF16, tag="ptile", bufs=4, name="pt"
                        )
                        nc.vector.tensor_scalar(
                            pt[:],
                            iota_s[:],
                            dln_s[:, ci : ci + 1],
                            None,
                            mybir.AluOpType.is_equal,
                        )
                        gi, c = p.chunk_call[ci]
                        gtile = (
                            gstate["tiles"][cbase + gi] if _PREP else call_tiles[gi]
                        )
                        nc.tensor.matmul(
                            out=agg[:],
                            lhsT=pt[:],
                            rhs=gtile[:, c : c + 1, :],
                            start=(k == 0),
                            stop=(k == nch - 1),
                        )
                    # epilogue: dst-side dis scale + bias (+ leaky relu)
                    t1 = wpool.tile([128, 128], F32, tag="ep1", name="t1")
                    nc.vector.tensor_scalar(
                        t1[:],
                        agg[:],
                        diso_s[:, b : b + 1],
                        None,
                        mybir.AluOpType.mult,
                    )
                    t2 = wpool.tile([128, 128], F32, tag="ep2", name="t2")
                    nc.vector.tensor_tensor(
                        out=t2[:], in0=t1[:], in1=bias_s[:], op=mybir.AluOpType.add
                    )
                    if layer == 1:
                        t3 = wpool.tile([128, 128], F32, tag="ep3", name="t3")
                        nc.vector.tensor_scalar(
                            t3[:], t2[:], NEG_SLOPE, None, mybir.AluOpType.mult
                        )
                        a1b = wpool.tile([128, 128], BF16, tag="a1b", name="a1b")
                        nc.vector.tensor_tensor(
                            out=a1b[:], in0=t2[:], in1=t3[:], op=mybir.AluOpType.max
                        )
                        tp = pspool.tile(
                            [128, 128], BF16, space="PSUM", tag="tp", name="tp"
                        )
                        nc.tensor.transpose(
                            out=tp[:], in_=a1b[:], identity=ident_s[:]
                        )
                        if b % 2 == 0:
                            nc.scalar.copy(
                                out=a1st[:, b * 128 : (b + 1) * 128], in_=tp[:]
                            )
                        else:
                            nc.vector.tensor_copy(
                                out=a1st[:, b * 128 : (b + 1) * 128], in_=tp[:]
                            )
                    else:
                        nc.vector.tensor_copy(out=outst[:, b, :], in_=t2[:])

                if _PREP:
                    flush_layer(len(all_calls))
                if layer == 1:
                    return a1st
                nc.sync.dma_start(
                    out=out_d.ap().rearrange("(b q) f -> q b f", q=128),
                    in_=outst[:, :, :],
                )
                return None

            def ag_phase(a1st):
                nc.sync.dma_start(out=agin_t[:, :], in_=a1st[:, :])
                nc.gpsimd.collective_compute(
                    "AllGather",
                    mybir.AluOpType.bypass,
                    replica_groups=[list(range(NCORES))],
                    ins=[agin_t[:, :].opt()],
                    outs=[agout_t[:, :].opt()],
                )

            h_phase(1)
            a1st = edge_phase(1)
            ag_phase(a1st)
            h_phase(2)
            edge_phase(2)

    nc.compile()
    return nc


_CACHE = {}


def _get_program(plan):
    nc = _CACHE.get(plan.key)
    if nc is None:
        nc = build_program(plan)
        _CACHE[plan.key] = nc
    return nc


def kernel(x, edge_index, batch, W1, b1, W2, b2):
    from concourse.bass_utils import run_bass_kernel_spmd

    x = np.asarray(x, np.float32)
    edge_index = np.asarray(edge_index)
    plan = make_plan(x.shape[0], edge_index)
    in_maps = make_in_maps(
        plan,
        x,
        np.asarray(W1, np.float32),
        np.asarray(b1, np.float32),
        np.asarray(W2, np.float32),
        np.asarray(b2, np.float32),
    )
    nc = _get_program(plan)
    res = run_bass_kernel_spmd(nc, in_maps, core_ids=list(range(NCORES)))
    out = np.concatenate([res.results[k]["out"] for k in range(NCORES)], axis=0)
    return np.ascontiguousarray(out[: plan.N]).astype(np.float32)
